# revision 1
# baseline (speedup 1.0000x reference)
"""Trainium2 Bass kernel for nn_Attention_39651138076722.

ChannelLayerNorm -> qkv 1x1 conv -> 4-head spatial attention (N=4096, dh=32)
-> proj 1x1 conv -> residual.   B=4, C=128, H=W=64.

Sharding: 8 cores = 4 batches x 2 head-pairs. Each core computes the partial
proj output of its 2 heads for its batch; the host sums the two partials.
LayerNorm affine (norm_w/norm_b) is folded into the qkv weights on the host.
Big matmuls run as float32r (~1.6e-4 max rel err, 4x faster than fp32).

Attention works on S^T = k.T q tiles [m=128, n=512]: four K=32 matmuls are
row-packed into the PE array per group (2 heads x 2 m-chunks, via
row-duplicated qq2/kk2 layouts), one big exp on ACT per 4-bank PSUM group,
and PV accumulates h rows + a ones-row (softmax denominator) per head.
Normalization + proj run as a deferred tail pass over saved h/denom tiles.
"""
import sys
sys.path.insert(0, "/opt/trn_rl_repo")

import numpy as np
import concourse.bass as bass
import concourse.tile as tile
from concourse import bacc, mybir
from concourse.bass_utils import run_bass_kernel_spmd

F32 = mybir.dt.float32
F32R = mybir.dt.float32r
AF = mybir.ActivationFunctionType
OP = mybir.AluOpType

B, C, H, W = 4, 128, 64, 64
N = H * W                      # 4096
NH, DH = 4, 32
EPS = 1e-6
NCH = 512                      # free-dim chunk (psum bank)
NJ = N // NCH                  # 8 n-chunks
MC = 128                       # m-chunk (partition tile)
NM = N // MC                   # 32 m-chunks
SCALE = DH ** -0.5


def build_nc(debug: bool = False, reps: int = 1):
    nc = bacc.Bacc("TRN2", target_bir_lowering=False)
    d_x = nc.dram_tensor("x", [C, N], F32, kind="ExternalInput")
    d_wqq = nc.dram_tensor("wqq", [C, 128], F32, kind="ExternalInput")
    d_wkk = nc.dram_tensor("wkk", [C, 128], F32, kind="ExternalInput")
    d_wv = nc.dram_tensor("wv", [C, 64], F32, kind="ExternalInput")
    d_bqq = nc.dram_tensor("bqq", [128, 1], F32, kind="ExternalInput")
    d_bkk = nc.dram_tensor("bkk", [128, 1], F32, kind="ExternalInput")
    d_bv = nc.dram_tensor("bv", [C, 64], F32, kind="ExternalInput")
    d_pw = nc.dram_tensor("pw", [65, C], F32, kind="ExternalInput")   # projT + bias row
    d_res = nc.dram_tensor("res", [C, 1], F32, kind="ExternalInput")  # residual scale col
    d_ind = nc.dram_tensor("ind", [2, 64], F32, kind="ExternalInput")  # head indicator
    d_out = nc.dram_tensor("out", [C, N], F32, kind="ExternalOutput")
    if debug:
        d_xhat = nc.dram_tensor("dbg_xhat", [C, N], F32, kind="ExternalOutput")
        d_qq = nc.dram_tensor("dbg_qq", [C, N], F32, kind="ExternalOutput")
        d_kk = nc.dram_tensor("dbg_kk", [C, N], F32, kind="ExternalOutput")
        d_vta = nc.dram_tensor("dbg_vta", [C, NM, 66], F32, kind="ExternalOutput")
        d_hb = nc.dram_tensor("dbg_hb", [64, N], F32, kind="ExternalOutput")
        d_rec = nc.dram_tensor("dbg_rec", [2, N], F32, kind="ExternalOutput")

    with tile.TileContext(nc) as tc:
        with tc.tile_pool(name="persist", bufs=1) as P:
            x_sb = P.tile([C, N], F32, tag="x_sb")
            xhat = P.tile([C, N], F32R, tag="xhat")
            qq2 = P.tile([C, N], F32R, tag="qq2")
            kk2 = P.tile([C, N], F32R, tag="kk2")
            vta = P.tile([C, NM, 66], F32R, tag="vta")   # per m-chunk: [v0|1|v1|1]
            w_qq = P.tile([C, 128], F32, tag="w_qq")
            w_kk = P.tile([C, 128], F32, tag="w_kk")
            w_v = P.tile([C, 64], F32, tag="w_v")
            wr_qq = P.tile([C, 128], F32R, tag="wr_qq")
            wr_kk = P.tile([C, 128], F32R, tag="wr_kk")
            wr_v = P.tile([C, 64], F32R, tag="wr_v")
            b_qq = P.tile([128, 1], F32, tag="b_qq")
            b_kk = P.tile([128, 1], F32, tag="b_kk")
            bv_b = P.tile([C, 64], F32, tag="bv_b")
            w_p = P.tile([65, C], F32, tag="w_p")
            wr_p = P.tile([65, C], F32R, tag="wr_p")
            res_c = P.tile([C, 1], F32, tag="res_c")
            ind2 = P.tile([2, 64], F32, tag="ind2")
            ones_m = P.tile([C, C], F32, tag="ones_m")
            ones_r = P.tile([C, C], F32R, tag="ones_r")
            eps_c = P.tile([C, 1], F32, tag="eps_c")
            ones_n = P.tile([1, NCH], F32, tag="ones_n")
            hn_a = P.tile([65, NCH], F32R, tag="hn_a")   # hn + ones row (proj bias)
            hn_b = P.tile([65, NCH], F32R, tag="hn_b")
            hbuf = [P.tile([64, NCH], F32, tag=f"hbuf{j}", name=f"hbuf{j}") for j in range(NJ)]
            rec01 = [P.tile([2, NCH], F32, tag=f"rec01_{j}", name=f"rec01_{j}") for j in range(NJ)]
            rec1t = [P.tile([1, NCH], F32, tag=f"rec1t_{j}", name=f"rec1t_{j}") for j in range(NJ)]

            nc.sync.dma_start(out=w_qq, in_=d_wqq.ap())
            nc.sync.dma_start(out=w_kk, in_=d_wkk.ap())
            nc.sync.dma_start(out=w_v, in_=d_wv.ap())
            nc.sync.dma_start(out=b_qq, in_=d_bqq.ap())
            nc.sync.dma_start(out=b_kk, in_=d_bkk.ap())
            nc.sync.dma_start(out=bv_b, in_=d_bv.ap())
            nc.sync.dma_start(out=w_p, in_=d_pw.ap())
            nc.sync.dma_start(out=res_c, in_=d_res.ap())
            nc.sync.dma_start(out=ind2, in_=d_ind.ap())
            nc.vector.memset(ones_m, 1.0)
            nc.vector.memset(eps_c, EPS)
            nc.vector.memset(ones_n, 1.0)
            nc.vector.tensor_copy(out=ones_r, in_=ones_m)
            nc.vector.tensor_copy(out=wr_qq, in_=w_qq)
            nc.vector.tensor_copy(out=wr_kk, in_=w_kk)
            nc.vector.tensor_copy(out=wr_v, in_=w_v)
            nc.vector.tensor_copy(out=wr_p, in_=w_p)
            nc.vector.tensor_copy(out=hn_a[64:65, :], in_=ones_n)
            nc.vector.tensor_copy(out=hn_b[64:65, :], in_=ones_n)
            nc.vector.tensor_copy(out=vta[:, :, 32:33], in_=ones_m[:, 0:NM])
            nc.vector.tensor_copy(out=vta[:, :, 65:66], in_=ones_m[:, 0:NM])

            for rep in range(reps):
                # ---------- Phase 1+2: x load, LN stats, xhat, projections ----
                with tc.tile_pool(name="stats", bufs=3) as SP, \
                     tc.tile_pool(name="spool", bufs=2, space="PSUM") as SPOOL, \
                     tc.tile_pool(name="pvpool", bufs=1, space="PSUM") as PVP, \
                     tc.tile_pool(name="dscr", bufs=1, space="DRAM") as DSC, \
                     tc.tile_pool(name="ptpool", bufs=3) as PTP, \
                     tc.tile_pool(name="opool", bufs=2) as OPO, \
                     tc.tile_pool(name="npool", bufs=2) as NPO:
                    scr = [DSC.tile([2, NCH], F32, tag=f"scr{j}", name=f"scr{j}")
                           for j in range(NJ)]
                    def tail_a(j):
                        rb = NPO.tile([64, NCH], F32, tag="rb", name=f"rb{j}")
                        src = bass.AP(tensor=scr[j].tensor, offset=scr[j].offset,
                                      ap=[[NCH, 2], [0, 32], [1, NCH]])
                        nc.sync.dma_start(out=rb, in_=src)
                        hn = hn_a if j % 2 == 0 else hn_b
                        nc.vector.tensor_mul(hn[0:32, :], hbuf[j][0:32, :],
                                             rb[0:32, :])
                        nc.vector.tensor_mul(hn[32:64, :], hbuf[j][32:64, :],
                                             rb[32:64, :])

                    def tail_b(j):
                        js = slice(j * NCH, (j + 1) * NCH)
                        hn = hn_a if j % 2 == 0 else hn_b
                        pj = SPOOL.tile([C, NCH], F32, tag="sg", name=f"pj{j}")
                        nc.tensor.matmul(pj, wr_p, hn, start=True, stop=True)
                        ot = OPO.tile([C, NCH], F32, tag="ot", name=f"ot{j}")
                        nc.vector.scalar_tensor_tensor(out=ot, in0=x_sb[:, js],
                                                       scalar=res_c, in1=pj,
                                                       op0=OP.mult, op1=OP.add)
                        nc.sync.dma_start(out=d_out.ap()[:, js], in_=ot)

                    pairs = [(i % 2, i // 2) for i in range(2 * NM)]
                    NG = (2 * NM + 2) // 3          # 22 groups per n-chunk

                    def attn_group(j, gi, pvs):
                        js = slice(j * NCH, (j + 1) * NCH)
                        g0 = 3 * gi
                        grp = pairs[g0:g0 + 3]
                        sg = SPOOL.tile([C, 3 * NCH], F32, tag="sg",
                                        name=f"sg{j}_{gi}")
                        seen = {0: 0, 1: 0}
                        for i, (h, mc) in enumerate(grp):
                            rg = h + 2 * seen[h]     # row-group 0..3
                            seen[h] += 1
                            ms = slice(mc * MC, (mc + 1) * MC)
                            rs = slice(rg * 32, (rg + 1) * 32)
                            nc.tensor.matmul(sg[:, i * NCH:(i + 1) * NCH],
                                             kk2[rs, ms], qq2[rs, js],
                                             start=True, stop=True,
                                             tile_position=(rg * 32, 0))
                        pt = PTP.tile([C, 3 * NCH], F32R, tag="pt",
                                      name=f"pt{j}_{gi}")
                        nw = len(grp) * NCH
                        nc.scalar.activation(out=pt[:, 0:nw], in_=sg[:, 0:nw],
                                             func=AF.Exp, scale=SCALE)
                        for i, (h, mc) in enumerate(grp):
                            pi = g0 + i
                            vcols = slice(33 * h, 33 * h + 33)
                            nc.tensor.matmul(pvs[h], vta[:, mc, vcols],
                                             pt[:, i * NCH:(i + 1) * NCH],
                                             start=(pi == h),
                                             stop=(pi == 2 * NM - 2 + h))

                    def attn_end(j, pvs):
                        pv0, pv1 = pvs
                        nc.vector.reciprocal(out=rec01[j][0:1, :],
                                             in_=pv0[32:33, :])
                        nc.vector.reciprocal(out=rec1t[j], in_=pv1[32:33, :])
                        nc.sync.dma_start(out=scr[j][0:1, :], in_=rec01[j][0:1, :])
                        nc.sync.dma_start(out=scr[j][1:2, :], in_=rec1t[j])
                        nc.vector.tensor_copy(out=hbuf[j][0:32, :],
                                              in_=pv0[0:32, :])
                        nc.vector.tensor_copy(out=hbuf[j][32:64, :],
                                              in_=pv1[0:32, :])


                    pv0_0 = PVP.tile([33, NCH], F32, tag="pv0", name="pv0_0")
                    pv1_0 = PVP.tile([33, NCH], F32, tag="pv1", name="pv1_0")
                    pvs0 = (pv0_0, pv1_0)
                    next_g = [0]

                    def stream_j0(c):
                        while next_g[0] < NG and \
                                min(3 * next_g[0] + 2, 2 * NM - 1) // 2 <= 4 * c + 3:
                            attn_group(0, next_g[0], pvs0)
                            next_g[0] += 1

                    mBs, invs, s12s = {}, {}, {}
                    for j in range(NJ):
                        js = slice(j * NCH, (j + 1) * NCH)
                        nc.sync.dma_start(out=x_sb[:, js], in_=d_x.ap()[:, js])
                        x2 = SP.tile([C, NCH], F32R, tag="x2", name=f"x2_{j}")
                        nc.vector.tensor_mul(x2, x_sb[:, js], x_sb[:, js])
                        s12 = SPOOL.tile([C, 2, NCH], F32, tag="sg", name=f"s12_{j}")
                        # all-ones lhsT => every out partition = sum over channels
                        nc.tensor.matmul(s12[:, 0, :], ones_m, x_sb[:, js],
                                         start=True, stop=True)
                        nc.tensor.matmul(s12[:, 1, :], ones_r, x2,
                                         start=True, stop=True)
                        mB = SP.tile([C, NCH], F32, tag="mB", name=f"mB_{j}")
                        nc.vector.tensor_scalar(out=mB, in0=s12[:, 0, :],
                                                scalar1=1.0 / C,
                                                scalar2=None, op0=OP.mult)
                        mBs[j] = mB
                        s12s[j] = s12[:, 1, :]
                        msq = SP.tile([C, NCH], F32, tag="msq", name=f"msq_{j}")
                        nc.vector.tensor_mul(msq, mB, mB)
                        var = SP.tile([C, NCH], F32, tag="var", name=f"var_{j}")
                        nc.vector.scalar_tensor_tensor(out=var, in0=s12s[j], scalar=1.0 / C,
                                                       in1=msq, op0=OP.mult,
                                                       op1=OP.subtract)
                        sd = SP.tile([C, NCH], F32, tag="sd", name=f"sd_{j}")
                        nc.scalar.activation(out=sd, in_=var, func=AF.Sqrt,
                                             bias=eps_c, scale=1.0)
                        inv = SP.tile([C, NCH], F32, tag="inv", name=f"inv_{j}")
                        nc.vector.reciprocal(out=inv, in_=sd)
                        invs[j] = inv
                    for j in range(NJ):
                        js = slice(j * NCH, (j + 1) * NCH)
                        cen = SP.tile([C, NCH], F32, tag="cen", name=f"cen_{j}")
                        nc.vector.tensor_sub(cen, x_sb[:, js], mBs[j])
                        nc.vector.tensor_mul(xhat[:, js], cen, invs[j])
                        # projections for this chunk
                        qkp = SPOOL.tile([C, 2, NCH], F32, tag="sg", name=f"qkp{j}")
                        nc.tensor.matmul(qkp[:, 0, :], wr_qq, xhat[:, js],
                                         start=True, stop=True)
                        nc.vector.tensor_scalar(out=qq2[:, js], in0=qkp[:, 0, :],
                                                scalar1=b_qq,
                                                scalar2=None, op0=OP.add)
                        nc.tensor.matmul(qkp[:, 1, :], wr_kk, xhat[:, js],
                                         start=True, stop=True)
                        nc.vector.tensor_scalar(out=kk2[:, js], in0=qkp[:, 1, :],
                                                scalar1=b_kk,
                                                scalar2=None, op0=OP.add)
                        vpq = SPOOL.tile([C, 4, 64], F32, tag="sg", name=f"vpq{j}")
                        for mq in range(4):
                            mc = 4 * j + mq
                            ms = slice(mc * MC, (mc + 1) * MC)
                            nc.tensor.matmul(vpq[:, mq, :], xhat[:, ms], wr_v,
                                             start=True, stop=True)
                            vdst = vta[:, mc, 0:66].rearrange(
                                "p (a b) -> p a b", a=2)[:, :, 0:32]
                            vsrc = vpq[:, mq, :].rearrange("p (a b) -> p a b", a=2)
                            bsrc = bv_b.rearrange("p (a b) -> p a b", a=2)
                            nc.vector.tensor_add(vdst, vsrc, bsrc)

                    stream_j0(NJ - 1)
                    attn_end(0, pvs0)
                    for j in range(1, NJ):
                        pv0 = PVP.tile([33, NCH], F32, tag="pv0", name=f"pv0_{j}")
                        pv1 = PVP.tile([33, NCH], F32, tag="pv1", name=f"pv1_{j}")
                        for gi in range(NG):
                            if gi == 3:
                                tail_a(j - 1)
                            if gi == 9:
                                tail_b(j - 1)
                            attn_group(j, gi, (pv0, pv1))
                        attn_end(j, (pv0, pv1))
                    tail_a(NJ - 1)
                    tail_b(NJ - 1)
    nc.compile()
    return nc


def _prep_inputs(x, norm_w, norm_b, qkv_w, qkv_b, proj_w, proj_b):
    """Host-side fold + per-core slicing. Returns list of 8 in_maps."""
    xf = np.ascontiguousarray(x.reshape(B, C, N), dtype=np.float32)
    qkv_wf = (qkv_w * norm_w[None, :]).astype(np.float32)
    qkv_bf = (qkv_b + qkv_w @ norm_b).astype(np.float32)
    in_maps = []
    for core in range(8):
        b, hp = core // 2, core % 2
        h0, h1 = 2 * hp, 2 * hp + 1
        qrows = list(range(h0 * DH, h0 * DH + DH)) + list(range(h1 * DH, h1 * DH + DH))
        krows = [C + r for r in qrows]
        vrows = [2 * C + r for r in qrows]
        qrows2 = qrows + qrows                           # duplicated for row-packing
        krows2 = krows + krows
        wqq = qkv_wf[qrows2, :].T.copy()                 # [C, 128]
        wkk = qkv_wf[krows2, :].T.copy()
        wv = qkv_wf[vrows, :].T.copy()                   # [C, 64]
        bqq = qkv_bf[qrows2].reshape(128, 1).copy()
        bkk = qkv_bf[krows2].reshape(128, 1).copy()
        bv = np.broadcast_to(qkv_bf[vrows].reshape(1, 64), (C, 64)).copy()
        cols = qrows
        pw = np.zeros((65, C), np.float32)
        pw[0:64, :] = proj_w[:, cols].T
        if hp == 0:
            pw[64, :] = proj_b
        res = np.full((C, 1), 1.0 if hp == 0 else 0.0, np.float32)
        ind = np.zeros((2, 64), np.float32)
        ind[0, 0:32] = 1.0
        ind[1, 32:64] = 1.0
        in_maps.append({
            "x": np.ascontiguousarray(xf[b]), "wqq": wqq, "wkk": wkk, "wv": wv,
            "bqq": bqq, "bkk": bkk, "bv": bv, "pw": pw, "res": res, "ind": ind,
        })
    return in_maps


_NC_CACHE = None


def kernel(x, norm_w, norm_b, qkv_w, qkv_b, proj_w, proj_b, **extra):
    global _NC_CACHE
    x = np.asarray(x, dtype=np.float32)
    in_maps = _prep_inputs(x, np.asarray(norm_w), np.asarray(norm_b),
                           np.asarray(qkv_w), np.asarray(qkv_b),
                           np.asarray(proj_w), np.asarray(proj_b))
    if _NC_CACHE is None:
        _NC_CACHE = build_nc()
    res = run_bass_kernel_spmd(_NC_CACHE, in_maps, core_ids=list(range(8)))
    parts = [res.results[i]["out"] for i in range(8)]
    out = np.empty((B, C, N), np.float32)
    for b in range(B):
        out[b] = parts[2 * b] + parts[2 * b + 1]
    return out.reshape(B, C, H, W)


if __name__ == "__main__":
    rng = np.random.default_rng(0)
    x = rng.standard_normal((B, C, H, W)).astype(np.float32)
    nw = np.ones(C, np.float32)
    nb = np.zeros(C, np.float32)
    qw = (rng.standard_normal((3 * C, C)) / np.sqrt(C)).astype(np.float32)
    qb = np.zeros(3 * C, np.float32)
    pw = (rng.standard_normal((C, C)) / np.sqrt(C)).astype(np.float32)
    pb = np.zeros(C, np.float32)
    got = kernel(x, nw, nb, qw, qb, pw, pb)
    print("kernel ran, shape", got.shape)



# revision 10
# speedup vs baseline: 1.0049x; 1.0049x over previous
"""Trainium2 Bass kernel for nn_Attention_39651138076722.

ChannelLayerNorm -> qkv 1x1 conv -> 4-head spatial attention (N=4096, dh=32)
-> proj 1x1 conv -> residual.   B=4, C=128, H=W=64.

Sharding: 8 cores = 4 batches x 2 head-pairs. Each core computes the partial
proj output of its 2 heads for its batch; the host sums the two partials.
LayerNorm affine (norm_w/norm_b) is folded into the qkv weights on the host.

Attention runs in fp8 with DoubleRow matmuls (0.5 PE cycles/row):
  S^T = k.T q: qq2/kk2 stored e4m3 as [64p, 2, N] (plane 1 zeroed), one
  DR matmul per (head, m-chunk) -> psum [128, 512].
  exp: per (head, m-chunk-pair), either ACT (native Exp -> e5m2) or DVE
  (Schraudolph: y = S*A + MAGIC in fp32; low byte of the fp32 result IS the
  e5m2 bit pattern of exp(S*scale), read via a stride-4 byte view).
  PV: per (m-pair, head) DR matmul with denominator ones col:
  lhsT = v2[128, 2, 33] e4m3 (plane stride padded to 80 for the 16B DR
  alignment rule), rhs = pt pair, out per-head psum [33, 512] at partition 0
  (DR matmuls must write partition 0) accumulated over the 16 pairs.
Normalization: reciprocal_approx_fast of pv rows, PE-broadcast via an
indicator matmul to rb66, one tensor_mul, then the fp32r proj tail.
"""
import sys
sys.path.insert(0, "/opt/trn_rl_repo")

import math
import numpy as np
import concourse.bass as bass
import concourse.tile as tile
from concourse import bacc, mybir
from concourse.bass_utils import run_bass_kernel_spmd

F32 = mybir.dt.float32
F32R = mybir.dt.float32r
F8E4 = mybir.dt.float8e4
F8E5 = mybir.dt.float8e5
AF = mybir.ActivationFunctionType
OP = mybir.AluOpType
PM = mybir.MatmulPerfMode

B, C, H, W = 4, 128, 64, 64
N = H * W                      # 4096
NH, DH = 4, 32
EPS = 1e-6
NCH = 512                      # free-dim chunk (psum bank)
NJ = N // NCH                  # 8 n-chunks
MC = 128                       # m-chunk (partition tile)
NM = N // MC                   # 32 m-chunks
NP = NM // 2                   # 16 m-chunk pairs
SCALE = DH ** -0.5
# Schraudolph exp -> e5m2 bits: byte0( round(4*log2e*scale*S + 60 - sigma) )
A_COEF = 4.0 * math.log2(math.e) * SCALE
MAGIC_B = 12582912.0 + (60.0 - 0.3)   # 1.5*2^23 + bias - sigma
# exp engine per m-pair: True -> ACT, False -> DVE  (10 ACT / 6 DVE)
ENG_ACT = [True, False, True, False, True, False, True, False,
           True, False, True, False, True, True, True, True]


def build_nc(debug: bool = False):
    nc = bacc.Bacc("TRN2", target_bir_lowering=False)
    d_x = nc.dram_tensor("x", [C, N], F32R, kind="ExternalInput")
    d_wqk = nc.dram_tensor("wqk", [C, 128], F32, kind="ExternalInput")
    d_wv = nc.dram_tensor("wv", [C, 64], F32, kind="ExternalInput")
    d_bq = nc.dram_tensor("bq", [64, 1], F32, kind="ExternalInput")
    d_bk = nc.dram_tensor("bk", [64, 1], F32, kind="ExternalInput")
    d_bv = nc.dram_tensor("bv", [C, 64], F32, kind="ExternalInput")
    d_pw = nc.dram_tensor("pw", [97, C], F32, kind="ExternalInput")   # projT+bias
    d_res = nc.dram_tensor("res", [C, 1], F32, kind="ExternalInput")  # residual col
    d_i97 = nc.dram_tensor("i97", [97, 97], F32, kind="ExternalInput")
    d_out = nc.dram_tensor("out", [C, N], F32, kind="ExternalOutput")
    if debug:
        d_qq = nc.dram_tensor("dbg_qq", [64, 2 * N], F8E4, kind="ExternalOutput")
        d_kk = nc.dram_tensor("dbg_kk", [64, 2 * N], F8E4, kind="ExternalOutput")
        d_v2 = nc.dram_tensor("dbg_v2", [128, NP * 2 * 80], F8E4,
                              kind="ExternalOutput")
        d_pt = nc.dram_tensor("dbg_pt", [128, 2 * NCH], F8E5, kind="ExternalOutput")
        d_ptd = nc.dram_tensor("dbg_ptd", [128, 2 * NCH], F32, kind="ExternalOutput")
        d_pv = nc.dram_tensor("dbg_pv", [97, NCH], F32, kind="ExternalOutput")
        d_rb = nc.dram_tensor("dbg_rb", [97, NCH], F32, kind="ExternalOutput")

    with tile.TileContext(nc) as tc:
        with tc.tile_pool(name="persist", bufs=1) as P:
            x_sb = P.tile([C, N], F32R, tag="x_sb")
            xhat = P.tile([C, N], F32R, tag="xhat")
            qq2 = P.tile([64, 2, N], F8E4, tag="qq2")
            kk2 = P.tile([64, 2, N], F8E4, tag="kk2")
            v2 = P.tile([128, NP, 2, 80], F8E4, tag="v2")
            w_qk = P.tile([C, 128], F32, tag="w_qk")
            w_v = P.tile([C, 64], F32, tag="w_v")
            wr_qk = P.tile([C, 128], F32R, tag="wr_qk")
            wr_v = P.tile([C, 64], F32R, tag="wr_v")
            b_q = P.tile([64, 1], F32, tag="b_q")
            b_k = P.tile([64, 1], F32, tag="b_k")
            bv_b = P.tile([C, 64], F32, tag="bv_b")
            w_p = P.tile([97, C], F32, tag="w_p")
            wr_p = P.tile([97, C], F32R, tag="wr_p")
            res_c = P.tile([C, 1], F32, tag="res_c")
            i97 = P.tile([97, 97], F32, tag="i97")
            ones_m = P.tile([C, C], F32, tag="ones_m")
            ones_r = P.tile([C, C], F32R, tag="ones_r")
            eps_c = P.tile([C, 1], F32, tag="eps_c")
            hn_a = P.tile([97, NCH], F32R, tag="hn_a")
            hn_b = P.tile([97, NCH], F32R, tag="hn_b")
            hb97 = [P.tile([97, NCH], F32, tag=f"hb97_{j}", name=f"hb97_{j}")
                    for j in range(NJ)]

            nc.sync.dma_start(out=w_qk, in_=d_wqk.ap())
            nc.sync.dma_start(out=w_v, in_=d_wv.ap())
            nc.sync.dma_start(out=b_q, in_=d_bq.ap())
            nc.sync.dma_start(out=b_k, in_=d_bk.ap())
            nc.sync.dma_start(out=bv_b, in_=d_bv.ap())
            nc.sync.dma_start(out=w_p, in_=d_pw.ap())
            nc.sync.dma_start(out=res_c, in_=d_res.ap())
            nc.sync.dma_start(out=i97, in_=d_i97.ap())
            nc.vector.memset(ones_m, 1.0)
            nc.vector.memset(eps_c, EPS)
            nc.vector.memset(qq2, 0.0)
            nc.vector.memset(kk2, 0.0)
            nc.vector.memset(v2, 0.0)
            nc.vector.memset(v2[:, :, :, 32:33], 1.0)
            nc.vector.memset(v2[:, :, :, 72:73], 1.0)
            nc.vector.tensor_copy(out=ones_r, in_=ones_m)
            nc.vector.tensor_copy(out=wr_qk, in_=w_qk)
            nc.vector.tensor_copy(out=wr_v, in_=w_v)
            nc.vector.tensor_copy(out=wr_p, in_=w_p)

            with tc.tile_pool(name="stats", bufs=3) as SP, \
                 tc.tile_pool(name="pspool", bufs=2, space="PSUM") as PS, \
                 tc.tile_pool(name="pvpool", bufs=1, space="PSUM") as PVP, \
                 tc.tile_pool(name="auxpool", bufs=1, space="PSUM") as AUX, \
                 tc.tile_pool(name="pta", bufs=3) as PA, \
                 tc.tile_pool(name="ptd", bufs=3) as PD, \
                 tc.tile_pool(name="opool", bufs=2) as OPO, \
                 tc.tile_pool(name="rpool", bufs=2) as RPO:

                pvs = {}

                def s_group(j, pair, h):
                    """S DR matmuls for both parities + exp -> returns pt AP."""
                    js = slice(j * NCH, (j + 1) * NCH)
                    hs = slice(32 * h, 32 * h + 32)
                    st = PS.tile([128, 2, NCH], F32, tag="st",
                                 name=f"st{j}_{pair}_{h}")
                    for par in (0, 1):
                        mc = 2 * pair + par
                        ms = slice(mc * MC, (mc + 1) * MC)
                        nc.tensor.matmul(st[:, par, :], kk2[hs, :, ms],
                                         qq2[hs, :, js], start=True, stop=True,
                                         perf_mode=PM.DoubleRow,
                                         tile_position=(32 * h, 0))
                    if ENG_ACT[pair]:
                        pt = PA.tile([128, 2, NCH], F8E5, tag="pta",
                                     name=f"pta{j}_{pair}_{h}")
                        nc.scalar.activation(out=pt, in_=st, func=AF.Exp,
                                             scale=SCALE)
                        rhs = pt[:, :, :]
                    else:
                        pt = PD.tile([128, 2, NCH], F32, tag="ptd",
                                     name=f"ptd{j}_{pair}_{h}")
                        nc.vector.tensor_scalar(out=pt, in0=st,
                                                scalar1=A_COEF, scalar2=MAGIC_B,
                                                op0=OP.mult, op1=OP.add)
                        rhs = pt[:, :, :].bitcast(F8E5).rearrange(
                            "p t (n f) -> p t n f", f=4)[:, :, :, 0:1]
                    return rhs

                def attn_pair(j, pair):
                    pv0, pv1 = pvs[j]
                    for h, pv in ((0, pv0), (1, pv1)):
                        rhs = s_group(j, pair, h)
                        vcols = slice(40 * h, 40 * h + 33)
                        nc.tensor.matmul(pv, v2[:, pair, :, vcols], rhs,
                                         start=(pair == 0), stop=(pair == NP - 1),
                                         perf_mode=PM.DoubleRow,
                                         tile_position=(0, 0))

                def tail_a(j):
                    # pv -> hb97 (ACT copies), reciprocal, rb broadcast matmul
                    pv0, pv1 = pvs.pop(j)
                    nc.scalar.activation(out=hb97[j][0:33, :], in_=pv0,
                                         func=AF.Copy)
                    nc.scalar.activation(out=hb97[j][64:97, :], in_=pv1,
                                         func=AF.Copy)
                    rec = RPO.tile([97, NCH], F32, tag="rec", name=f"rec{j}")
                    nc.vector.reciprocal_approx_fast(out=rec, in_=hb97[j])
                    rb = AUX.tile([97, NCH], F32, tag="rb", name=f"rb{j}")
                    nc.tensor.matmul(rb, i97, rec, start=True, stop=True)
                    return rb

                rbs = {}

                def tail_b(j):
                    js = slice(j * NCH, (j + 1) * NCH)
                    rb = rbs.pop(j)
                    hn = hn_a if j % 2 == 0 else hn_b
                    nc.vector.tensor_mul(hn, hb97[j], rb)
                    pj = AUX.tile([C, NCH], F32, tag="pj", name=f"pj{j}")
                    nc.tensor.matmul(pj, wr_p, hn, start=True, stop=True)
                    ot = OPO.tile([C, NCH], F32, tag="ot", name=f"ot{j}")
                    nc.vector.scalar_tensor_tensor(out=ot, in0=x_sb[:, js],
                                                   scalar=res_c, in1=pj,
                                                   op0=OP.mult, op1=OP.add)
                    nc.sync.dma_start(out=d_out.ap()[:, js], in_=ot)

                # ---------------- phase 1 + streamed nc0 ---------------------
                pvs[0] = (PVP.tile([33, NCH], F32, tag="pv0", name="pv0_0"),
                          PVP.tile([33, NCH], F32, tag="pv1", name="pv1_0"))
                next_p = [0]

                def stream_j0(c):
                    while next_p[0] < NP and next_p[0] <= 2 * c + 1:
                        attn_pair(0, next_p[0])
                        next_p[0] += 1

                for c in range(NJ):
                    cs = slice(c * NCH, (c + 1) * NCH)
                    nc.sync.dma_start(out=x_sb[:, cs], in_=d_x.ap()[:, cs])
                    x2 = SP.tile([C, NCH], F32R, tag="x2", name=f"x2_{c}")
                    nc.vector.tensor_mul(x2, x_sb[:, cs], x_sb[:, cs])
                    s12 = PS.tile([128, 2, NCH], F32, tag="st", name=f"s12_{c}")
                    nc.tensor.matmul(s12[:, 0, :], ones_r, x_sb[:, cs],
                                     start=True, stop=True)
                    nc.tensor.matmul(s12[:, 1, :], ones_r, x2,
                                     start=True, stop=True)
                    mB = SP.tile([C, NCH], F32, tag="mB", name=f"mB_{c}")
                    nc.vector.tensor_scalar(out=mB, in0=s12[:, 0, :],
                                            scalar1=1.0 / C, scalar2=None,
                                            op0=OP.mult)
                    msq = SP.tile([C, NCH], F32, tag="msq", name=f"msq_{c}")
                    nc.vector.tensor_mul(msq, mB, mB)
                    var = SP.tile([C, NCH], F32, tag="var", name=f"var_{c}")
                    nc.vector.scalar_tensor_tensor(out=var, in0=s12[:, 1, :],
                                                   scalar=1.0 / C, in1=msq,
                                                   op0=OP.mult, op1=OP.subtract)
                    sd = SP.tile([C, NCH], F32, tag="sd", name=f"sd_{c}")
                    nc.scalar.activation(out=sd, in_=var, func=AF.Sqrt,
                                         bias=eps_c, scale=1.0)
                    inv = SP.tile([C, NCH], F32, tag="inv", name=f"inv_{c}")
                    nc.vector.reciprocal_approx_fast(out=inv, in_=sd)
                    cen = SP.tile([C, NCH], F32, tag="cen", name=f"cen_{c}")
                    nc.vector.tensor_sub(cen, x_sb[:, cs], mB)
                    nc.vector.tensor_mul(xhat[:, cs], cen, inv)

                    qkp = PS.tile([128, 2, NCH], F32, tag="st", name=f"qkp_{c}")
                    nc.tensor.matmul(qkp[:, 0, :], wr_qk, xhat[:, cs],
                                     start=True, stop=True)
                    nc.vector.tensor_scalar(out=qq2[:, 0, cs], in0=qkp[0:64, 0, :],
                                            scalar1=b_q, scalar2=None, op0=OP.add)
                    nc.vector.tensor_scalar(out=kk2[:, 0, cs], in0=qkp[64:128, 0, :],
                                            scalar1=b_k, scalar2=None, op0=OP.add)
                    vpq = qkp[:, 1, 0:256].rearrange("p (a b) -> p a b", a=4)
                    for mq in range(4):
                        mc = 4 * c + mq
                        ms = slice(mc * MC, (mc + 1) * MC)
                        nc.tensor.matmul(vpq[:, mq, :], xhat[:, ms], wr_v,
                                         start=True, stop=True)
                        vdst = v2[:, mc // 2, mc % 2, :].rearrange(
                            "p (h x) -> p h x", h=2)[:, :, 0:32]
                        vsrc = vpq[:, mq, :].rearrange("p (a b) -> p a b", a=2)
                        bsrc = bv_b.rearrange("p (a b) -> p a b", a=2)
                        nc.vector.tensor_add(vdst, vsrc, bsrc)
                    stream_j0(c)

                if debug:
                    nc.sync.dma_start(out=d_qq.ap(),
                                      in_=qq2[:, :, :].rearrange("p t n -> p (t n)"))
                    nc.sync.dma_start(out=d_kk.ap(),
                                      in_=kk2[:, :, :].rearrange("p t n -> p (t n)"))
                    nc.sync.dma_start(
                        out=d_v2.ap(),
                        in_=v2[:, :, :, :].rearrange("p a t n -> p (a t n)"))

                # ---------------- n-chunks 1..7 with deferred tails ----------
                for j in range(1, NJ):
                    rbs[j - 1] = tail_a(j - 1)
                    pvs[j] = (PVP.tile([33, NCH], F32, tag="pv0", name=f"pv0_{j}"),
                              PVP.tile([33, NCH], F32, tag="pv1", name=f"pv1_{j}"))
                    for pair in range(NP):
                        if pair == 6:
                            tail_b(j - 1)
                        attn_pair(j, pair)
                if debug:
                    nc.sync.dma_start(out=d_pv.ap(), in_=hb97[NJ - 1][:, :])
                rbs[NJ - 1] = tail_a(NJ - 1)
                if debug:
                    nc.sync.dma_start(out=d_rb.ap(), in_=rbs[NJ - 1][0:97, :])
                tail_b(NJ - 1)
    nc.compile()
    return nc


def _prep_inputs(x, norm_w, norm_b, qkv_w, qkv_b, proj_w, proj_b):
    """Host-side fold + per-core slicing. Returns list of 8 in_maps."""
    xf = np.ascontiguousarray(x.reshape(B, C, N), dtype=np.float32)
    qkv_wf = (qkv_w * norm_w[None, :]).astype(np.float32)
    qkv_bf = (qkv_b + qkv_w @ norm_b).astype(np.float32)
    i97 = np.zeros((97, 97), np.float32)
    i97[32, 0:32] = 1.0
    i97[96, 64:96] = 1.0
    i97[96, 96] = 1.0    # hn[96] = denom1 * (1/denom1) = 1 -> proj bias row
    in_maps = []
    for core in range(8):
        b, hp = core // 2, core % 2
        h0, h1 = 2 * hp, 2 * hp + 1
        qrows = list(range(h0 * DH, h0 * DH + DH)) + \
            list(range(h1 * DH, h1 * DH + DH))
        krows = [C + r for r in qrows]
        vrows = [2 * C + r for r in qrows]
        wqk = np.concatenate([qkv_wf[qrows, :].T,
                              qkv_wf[krows, :].T], axis=1).copy()  # [C, 128]
        wv = qkv_wf[vrows, :].T.copy()
        bq = qkv_bf[qrows].reshape(64, 1).copy()
        bk = qkv_bf[krows].reshape(64, 1).copy()
        bv = np.broadcast_to(qkv_bf[vrows].reshape(1, 64), (C, 64)).copy()
        pw = np.zeros((97, C), np.float32)
        pw[0:32, :] = proj_w[:, h0 * DH:h0 * DH + DH].T
        pw[64:96, :] = proj_w[:, h1 * DH:h1 * DH + DH].T
        if hp == 0:
            pw[96, :] = proj_b
        res = np.full((C, 1), 1.0 if hp == 0 else 0.0, np.float32)
        in_maps.append({
            "x": np.ascontiguousarray(xf[b]), "wqk": wqk, "wv": wv,
            "bq": bq, "bk": bk, "bv": bv, "pw": pw, "res": res, "i97": i97,
        })
    return in_maps


_NC_CACHE = None


def kernel(x, norm_w, norm_b, qkv_w, qkv_b, proj_w, proj_b, **extra):
    global _NC_CACHE
    x = np.asarray(x, dtype=np.float32)
    in_maps = _prep_inputs(x, np.asarray(norm_w), np.asarray(norm_b),
                           np.asarray(qkv_w), np.asarray(qkv_b),
                           np.asarray(proj_w), np.asarray(proj_b))
    if _NC_CACHE is None:
        _NC_CACHE = build_nc()
    res = run_bass_kernel_spmd(_NC_CACHE, in_maps, core_ids=list(range(8)))
    parts = [res.results[i]["out"] for i in range(8)]
    out = np.empty((B, C, N), np.float32)
    for b in range(B):
        out[b] = parts[2 * b] + parts[2 * b + 1]
    return out.reshape(B, C, H, W)


if __name__ == "__main__":
    rng = np.random.default_rng(0)
    x = rng.standard_normal((B, C, H, W)).astype(np.float32)
    nw = np.ones(C, np.float32)
    nb = np.zeros(C, np.float32)
    qw = (rng.standard_normal((3 * C, C)) / np.sqrt(C)).astype(np.float32)
    qb = np.zeros(3 * C, np.float32)
    pw = (rng.standard_normal((C, C)) / np.sqrt(C)).astype(np.float32)
    pb = np.zeros(C, np.float32)
    got = kernel(x, nw, nb, qw, qb, pw, pb)
    print("kernel ran, shape", got.shape)


# revision 18
# speedup vs baseline: 1.2549x; 1.2488x over previous
"""Trainium2 Bass kernel for nn_Attention_39651138076722.

ChannelLayerNorm -> qkv 1x1 conv -> 4-head spatial attention (N=4096, dh=32)
-> proj 1x1 conv -> residual.   B=4, C=128, H=W=64.

Sharding: 8 cores = 4 batches x 2 head-pairs. Each core computes the partial
proj output of its 2 heads for its batch; the host sums the two partials.
LayerNorm affine (norm_w/norm_b) is folded into the qkv weights on the host.
Big matmuls run as float32r (~1.6e-4 max rel err, 4x faster than fp32).

Attention works on S^T = k.T q tiles [m=128, n=512]: four K=32 matmuls are
row-packed into the PE array per group (2 heads x 2 m-chunks, via
row-duplicated qq2/kk2 layouts), one big exp on ACT per 4-bank PSUM group,
and PV accumulates h rows + a ones-row (softmax denominator) per head.
Normalization + proj run as a deferred tail pass over saved h/denom tiles.
Reciprocals use the single-pass approx-fast DVE op (~18 correct bits).
"""
import sys
sys.path.insert(0, "/opt/trn_rl_repo")

import numpy as np
import concourse.bass as bass
import concourse.tile as tile
from concourse import bacc, mybir
from concourse.bass_utils import run_bass_kernel_spmd

F32 = mybir.dt.float32
F32R = mybir.dt.float32r
AF = mybir.ActivationFunctionType
OP = mybir.AluOpType

B, C, H, W = 4, 128, 64, 64
N = H * W                      # 4096
NH, DH = 4, 32
EPS = 1e-6
NCH = 512                      # free-dim chunk (psum bank)
NJ = N // NCH                  # 8 n-chunks
MC = 128                       # m-chunk (partition tile)
NM = N // MC                   # 32 m-chunks
SCALE = DH ** -0.5


def build_nc(debug: bool = False, reps: int = 1):
    nc = bacc.Bacc("TRN2", target_bir_lowering=False)
    d_x = nc.dram_tensor("x", [C, N], F32, kind="ExternalInput")
    d_wqq = nc.dram_tensor("wqq", [C, 128], F32, kind="ExternalInput")
    d_wkk = nc.dram_tensor("wkk", [C, 128], F32, kind="ExternalInput")
    d_wv = nc.dram_tensor("wv", [C, 64], F32, kind="ExternalInput")
    d_bqq = nc.dram_tensor("bqq", [128, 1], F32, kind="ExternalInput")
    d_bkk = nc.dram_tensor("bkk", [128, 1], F32, kind="ExternalInput")
    d_bv = nc.dram_tensor("bv", [C, 64], F32, kind="ExternalInput")
    d_pw = nc.dram_tensor("pw", [65, C], F32, kind="ExternalInput")   # projT + bias row
    d_res = nc.dram_tensor("res", [C, 1], F32, kind="ExternalInput")  # residual scale col
    d_ind = nc.dram_tensor("ind", [2, 64], F32, kind="ExternalInput")  # head indicator
    d_out = nc.dram_tensor("out", [C, N], F32, kind="ExternalOutput")

    with tile.TileContext(nc) as tc:
        with tc.tile_pool(name="persist", bufs=1) as P:
            x_sb = P.tile([C, N], F32, tag="x_sb")
            xhat = P.tile([C, N], F32R, tag="xhat")
            qq2 = P.tile([C, N], F32R, tag="qq2")
            kk2 = P.tile([C, N], F32R, tag="kk2")
            vta = P.tile([C, NM, 66], F32R, tag="vta")   # per m-chunk: [v0|1|v1|1]
            w_qq = P.tile([C, 128], F32, tag="w_qq")
            w_kk = P.tile([C, 128], F32, tag="w_kk")
            w_v = P.tile([C, 64], F32, tag="w_v")
            wr_qq = P.tile([C, 128], F32R, tag="wr_qq")
            wr_kk = P.tile([C, 128], F32R, tag="wr_kk")
            wr_v = P.tile([C, 64], F32R, tag="wr_v")
            b_qq = P.tile([128, 1], F32, tag="b_qq")
            b_kk = P.tile([128, 1], F32, tag="b_kk")
            bv_b = P.tile([C, 64], F32, tag="bv_b")
            w_p = P.tile([65, C], F32, tag="w_p")
            wr_p = P.tile([65, C], F32R, tag="wr_p")
            res_c = P.tile([C, 1], F32, tag="res_c")
            ind2 = P.tile([2, 64], F32, tag="ind2")
            ones_m = P.tile([C, C], F32, tag="ones_m")
            ones_r = P.tile([C, C], F32R, tag="ones_r")
            eps_c = P.tile([C, 1], F32, tag="eps_c")
            ones_n = P.tile([1, NCH], F32, tag="ones_n")
            hn_a = P.tile([65, NCH], F32R, tag="hn_a")   # hn + ones row (proj bias)
            hn_b = P.tile([65, NCH], F32R, tag="hn_b")
            hbuf = [P.tile([64, NCH], F32, tag=f"hbuf{j}", name=f"hbuf{j}") for j in range(NJ)]
            rec01 = [P.tile([2, NCH], F32, tag=f"rec01_{j}", name=f"rec01_{j}") for j in range(NJ)]
            rec1t = [P.tile([1, NCH], F32, tag=f"rec1t_{j}", name=f"rec1t_{j}") for j in range(NJ)]

            nc.sync.dma_start(out=w_qq, in_=d_wqq.ap())
            nc.sync.dma_start(out=w_kk, in_=d_wkk.ap())
            nc.sync.dma_start(out=w_v, in_=d_wv.ap())
            nc.sync.dma_start(out=b_qq, in_=d_bqq.ap())
            nc.sync.dma_start(out=b_kk, in_=d_bkk.ap())
            nc.sync.dma_start(out=bv_b, in_=d_bv.ap())
            nc.sync.dma_start(out=w_p, in_=d_pw.ap())
            nc.sync.dma_start(out=res_c, in_=d_res.ap())
            nc.sync.dma_start(out=ind2, in_=d_ind.ap())
            nc.vector.memset(ones_m, 1.0)
            nc.vector.memset(eps_c, EPS)
            nc.vector.memset(ones_n, 1.0)
            nc.vector.tensor_copy(out=ones_r, in_=ones_m)
            nc.vector.tensor_copy(out=wr_qq, in_=w_qq)
            nc.vector.tensor_copy(out=wr_kk, in_=w_kk)
            nc.vector.tensor_copy(out=wr_v, in_=w_v)
            nc.vector.tensor_copy(out=wr_p, in_=w_p)
            nc.vector.tensor_copy(out=hn_a[64:65, :], in_=ones_n)
            nc.vector.tensor_copy(out=hn_b[64:65, :], in_=ones_n)
            nc.vector.tensor_copy(out=vta[:, :, 32:33], in_=ones_m[:, 0:NM])
            nc.vector.tensor_copy(out=vta[:, :, 65:66], in_=ones_m[:, 0:NM])

            for rep in range(reps):
                # ---------- Phase 1+2: x load, LN stats, xhat, projections ----
                with tc.tile_pool(name="stats", bufs=3) as SP, \
                     tc.tile_pool(name="spool", bufs=2, space="PSUM") as SPOOL, \
                     tc.tile_pool(name="pvpool", bufs=1, space="PSUM") as PVP, \
                     tc.tile_pool(name="dscr", bufs=1, space="DRAM") as DSC, \
                     tc.tile_pool(name="ptpool", bufs=3) as PTP, \
                     tc.tile_pool(name="opool", bufs=2) as OPO, \
                     tc.tile_pool(name="npool", bufs=2) as NPO:
                    scr = [DSC.tile([2, NCH], F32, tag=f"scr{j}", name=f"scr{j}")
                           for j in range(NJ)]
                    def tail_a(j):
                        rb = NPO.tile([64, NCH], F32, tag="rb", name=f"rb{j}")
                        src = bass.AP(tensor=scr[j].tensor, offset=scr[j].offset,
                                      ap=[[NCH, 2], [0, 32], [1, NCH]])
                        nc.sync.dma_start(out=rb, in_=src)
                        hn = hn_a if j % 2 == 0 else hn_b
                        nc.vector.tensor_mul(hn[0:32, :], hbuf[j][0:32, :],
                                             rb[0:32, :])
                        nc.vector.tensor_mul(hn[32:64, :], hbuf[j][32:64, :],
                                             rb[32:64, :])

                    def tail_b(j):
                        js = slice(j * NCH, (j + 1) * NCH)
                        hn = hn_a if j % 2 == 0 else hn_b
                        pj = SPOOL.tile([C, NCH], F32, tag="sg", name=f"pj{j}")
                        nc.tensor.matmul(pj, wr_p, hn, start=True, stop=True)
                        ot = OPO.tile([C, NCH], F32, tag="ot", name=f"ot{j}")
                        nc.vector.scalar_tensor_tensor(out=ot, in0=x_sb[:, js],
                                                       scalar=res_c, in1=pj,
                                                       op0=OP.mult, op1=OP.add)
                        nc.sync.dma_start(out=d_out.ap()[:, js], in_=ot)

                    pairs = [(i % 2, i // 2) for i in range(2 * NM)]
                    NG = (2 * NM + 2) // 3          # 22 groups per n-chunk

                    def attn_group(j, gi, pvs):
                        js = slice(j * NCH, (j + 1) * NCH)
                        g0 = 3 * gi
                        grp = pairs[g0:g0 + 3]
                        sg = SPOOL.tile([C, 3 * NCH], F32, tag="sg",
                                        name=f"sg{j}_{gi}")
                        seen = {0: 0, 1: 0}
                        for i, (h, mc) in enumerate(grp):
                            rg = h + 2 * seen[h]     # row-group 0..3
                            seen[h] += 1
                            ms = slice(mc * MC, (mc + 1) * MC)
                            rs = slice(rg * 32, (rg + 1) * 32)
                            nc.tensor.matmul(sg[:, i * NCH:(i + 1) * NCH],
                                             kk2[rs, ms], qq2[rs, js],
                                             start=True, stop=True,
                                             tile_position=(rg * 32, 0))
                        pt = PTP.tile([C, 3 * NCH], F32R, tag="pt",
                                      name=f"pt{j}_{gi}")
                        nw = len(grp) * NCH
                        nc.scalar.activation(out=pt[:, 0:nw], in_=sg[:, 0:nw],
                                             func=AF.Exp, scale=SCALE)
                        for i, (h, mc) in enumerate(grp):
                            pi = g0 + i
                            vcols = slice(33 * h, 33 * h + 33)
                            nc.tensor.matmul(pvs[h], vta[:, mc, vcols],
                                             pt[:, i * NCH:(i + 1) * NCH],
                                             start=(pi == h),
                                             stop=(pi == 2 * NM - 2 + h))

                    def attn_end(j, pvs):
                        pv0, pv1 = pvs
                        nc.vector.reciprocal(out=rec01[j][0:1, :],
                                             in_=pv0[32:33, :])
                        nc.vector.reciprocal(out=rec1t[j], in_=pv1[32:33, :])
                        nc.sync.dma_start(out=scr[j][0:1, :], in_=rec01[j][0:1, :])
                        nc.sync.dma_start(out=scr[j][1:2, :], in_=rec1t[j])
                        nc.vector.tensor_copy(out=hbuf[j][0:32, :],
                                              in_=pv0[0:32, :])
                        nc.vector.tensor_copy(out=hbuf[j][32:64, :],
                                              in_=pv1[0:32, :])


                    pv0_0 = PVP.tile([33, NCH], F32, tag="pv0", name="pv0_0")
                    pv1_0 = PVP.tile([33, NCH], F32, tag="pv1", name="pv1_0")
                    pvs0 = (pv0_0, pv1_0)
                    next_g = [0]

                    def stream_j0(c):
                        while next_g[0] < NG and \
                                min(3 * next_g[0] + 2, 2 * NM - 1) // 2 <= 4 * c + 3:
                            attn_group(0, next_g[0], pvs0)
                            next_g[0] += 1

                    mBs, invs, s12s = {}, {}, {}
                    for j in range(NJ):
                        js = slice(j * NCH, (j + 1) * NCH)
                        nc.sync.dma_start(out=x_sb[:, js], in_=d_x.ap()[:, js])
                        x2 = SP.tile([C, NCH], F32R, tag="x2", name=f"x2_{j}")
                        nc.vector.tensor_mul(x2, x_sb[:, js], x_sb[:, js])
                        s12 = SPOOL.tile([C, 2, NCH], F32, tag="sg", name=f"s12_{j}")
                        # all-ones lhsT => every out partition = sum over channels
                        nc.tensor.matmul(s12[:, 0, :], ones_m, x_sb[:, js],
                                         start=True, stop=True)
                        nc.tensor.matmul(s12[:, 1, :], ones_r, x2,
                                         start=True, stop=True)
                        mB = SP.tile([C, NCH], F32, tag="mB", name=f"mB_{j}")
                        nc.vector.tensor_scalar(out=mB, in0=s12[:, 0, :],
                                                scalar1=1.0 / C,
                                                scalar2=None, op0=OP.mult)
                        mBs[j] = mB
                        s12s[j] = s12[:, 1, :]
                        msq = SP.tile([C, NCH], F32, tag="msq", name=f"msq_{j}")
                        nc.vector.tensor_mul(msq, mB, mB)
                        var = SP.tile([C, NCH], F32, tag="var", name=f"var_{j}")
                        nc.vector.scalar_tensor_tensor(out=var, in0=s12s[j], scalar=1.0 / C,
                                                       in1=msq, op0=OP.mult,
                                                       op1=OP.subtract)
                        sd = SP.tile([C, NCH], F32, tag="sd", name=f"sd_{j}")
                        nc.scalar.activation(out=sd, in_=var, func=AF.Sqrt,
                                             bias=eps_c, scale=1.0)
                        inv = SP.tile([C, NCH], F32, tag="inv", name=f"inv_{j}")
                        nc.vector.reciprocal_approx_fast(out=inv, in_=sd)
                        invs[j] = inv
                    for j in range(NJ):
                        js = slice(j * NCH, (j + 1) * NCH)
                        cen = SP.tile([C, NCH], F32, tag="cen", name=f"cen_{j}")
                        nc.vector.tensor_sub(cen, x_sb[:, js], mBs[j])
                        nc.vector.tensor_mul(xhat[:, js], cen, invs[j])
                        # projections for this chunk
                        qkp = SPOOL.tile([C, 2, NCH], F32, tag="sg", name=f"qkp{j}")
                        nc.tensor.matmul(qkp[:, 0, :], wr_qq, xhat[:, js],
                                         start=True, stop=True)
                        nc.vector.tensor_scalar(out=qq2[:, js], in0=qkp[:, 0, :],
                                                scalar1=b_qq,
                                                scalar2=None, op0=OP.add)
                        nc.tensor.matmul(qkp[:, 1, :], wr_kk, xhat[:, js],
                                         start=True, stop=True)
                        nc.vector.tensor_scalar(out=kk2[:, js], in0=qkp[:, 1, :],
                                                scalar1=b_kk,
                                                scalar2=None, op0=OP.add)
                        vpq = SPOOL.tile([C, 4, 64], F32, tag="sg", name=f"vpq{j}")
                        for mq in range(4):
                            mc = 4 * j + mq
                            ms = slice(mc * MC, (mc + 1) * MC)
                            nc.tensor.matmul(vpq[:, mq, :], xhat[:, ms], wr_v,
                                             start=True, stop=True)
                            vdst = vta[:, mc, 0:66].rearrange(
                                "p (a b) -> p a b", a=2)[:, :, 0:32]
                            vsrc = vpq[:, mq, :].rearrange("p (a b) -> p a b", a=2)
                            bsrc = bv_b.rearrange("p (a b) -> p a b", a=2)
                            nc.vector.tensor_add(vdst, vsrc, bsrc)

                    stream_j0(NJ - 1)
                    attn_end(0, pvs0)
                    for j in range(1, NJ):
                        pv0 = PVP.tile([33, NCH], F32, tag="pv0", name=f"pv0_{j}")
                        pv1 = PVP.tile([33, NCH], F32, tag="pv1", name=f"pv1_{j}")
                        for gi in range(NG):
                            if gi == 3:
                                tail_a(j - 1)
                            if gi == 9:
                                tail_b(j - 1)
                            attn_group(j, gi, (pv0, pv1))
                        attn_end(j, (pv0, pv1))
                    tail_a(NJ - 1)
                    tail_b(NJ - 1)
    nc.compile()
    return nc


def _prep_inputs(x, norm_w, norm_b, qkv_w, qkv_b, proj_w, proj_b):
    """Host-side fold + per-core slicing. Returns list of 8 in_maps."""
    xf = np.ascontiguousarray(x.reshape(B, C, N), dtype=np.float32)
    qkv_wf = (qkv_w * norm_w[None, :]).astype(np.float32)
    qkv_bf = (qkv_b + qkv_w @ norm_b).astype(np.float32)
    in_maps = []
    for core in range(8):
        b, hp = core // 2, core % 2
        h0, h1 = 2 * hp, 2 * hp + 1
        qrows = list(range(h0 * DH, h0 * DH + DH)) + list(range(h1 * DH, h1 * DH + DH))
        krows = [C + r for r in qrows]
        vrows = [2 * C + r for r in qrows]
        qrows2 = qrows + qrows                           # duplicated for row-packing
        krows2 = krows + krows
        wqq = qkv_wf[qrows2, :].T.copy()                 # [C, 128]
        wkk = qkv_wf[krows2, :].T.copy()
        wv = qkv_wf[vrows, :].T.copy()                   # [C, 64]
        bqq = qkv_bf[qrows2].reshape(128, 1).copy()
        bkk = qkv_bf[krows2].reshape(128, 1).copy()
        bv = np.broadcast_to(qkv_bf[vrows].reshape(1, 64), (C, 64)).copy()
        cols = qrows
        pw = np.zeros((65, C), np.float32)
        pw[0:64, :] = proj_w[:, cols].T
        if hp == 0:
            pw[64, :] = proj_b
        res = np.full((C, 1), 1.0 if hp == 0 else 0.0, np.float32)
        ind = np.zeros((2, 64), np.float32)
        ind[0, 0:32] = 1.0
        ind[1, 32:64] = 1.0
        in_maps.append({
            "x": np.ascontiguousarray(xf[b]), "wqq": wqq, "wkk": wkk, "wv": wv,
            "bqq": bqq, "bkk": bkk, "bv": bv, "pw": pw, "res": res, "ind": ind,
        })
    return in_maps


_NC_CACHE = None


def kernel(x, norm_w, norm_b, qkv_w, qkv_b, proj_w, proj_b, **extra):
    global _NC_CACHE
    x = np.asarray(x, dtype=np.float32)
    in_maps = _prep_inputs(x, np.asarray(norm_w), np.asarray(norm_b),
                           np.asarray(qkv_w), np.asarray(qkv_b),
                           np.asarray(proj_w), np.asarray(proj_b))
    if _NC_CACHE is None:
        _NC_CACHE = build_nc()
    res = run_bass_kernel_spmd(_NC_CACHE, in_maps, core_ids=list(range(8)))
    parts = [res.results[i]["out"] for i in range(8)]
    out = np.empty((B, C, N), np.float32)
    for b in range(B):
        out[b] = parts[2 * b] + parts[2 * b + 1]
    return out.reshape(B, C, H, W)


if __name__ == "__main__":
    rng = np.random.default_rng(0)
    x = rng.standard_normal((B, C, H, W)).astype(np.float32)
    nw = np.ones(C, np.float32)
    nb = np.zeros(C, np.float32)
    qw = (rng.standard_normal((3 * C, C)) / np.sqrt(C)).astype(np.float32)
    qb = np.zeros(3 * C, np.float32)
    pw = (rng.standard_normal((C, C)) / np.sqrt(C)).astype(np.float32)
    pb = np.zeros(C, np.float32)
    got = kernel(x, nw, nb, qw, qb, pw, pb)
    print("kernel ran, shape", got.shape)


# revision 20
# speedup vs baseline: 1.4201x; 1.1317x over previous
"""Trainium2 Bass kernel for nn_Attention_39651138076722.

ChannelLayerNorm -> qkv 1x1 conv -> 4-head spatial attention (N=4096, dh=32)
-> proj 1x1 conv -> residual.   B=4, C=128, H=W=64.

Sharding: 8 cores = 4 batches x 2 head-pairs. Each core computes the partial
proj output of its 2 heads for its batch; the host sums the two partials.
LayerNorm affine (norm_w/norm_b) is folded into the qkv weights on the host.
Big matmuls run as float32r (~1.6e-4 max rel err, 4x faster than fp32).

Attention works on S^T = k.T q tiles [m=128, n=512]: four K=32 matmuls are
row-packed into the PE array per group (2 heads x 2 m-chunks, via
row-duplicated qq2/kk2 layouts), one big exp on ACT per 4-bank PSUM group,
and PV accumulates h rows + a ones-row (softmax denominator) per head.
Normalization + proj run as a deferred tail pass over saved h/denom tiles.
Reciprocals use the single-pass approx-fast DVE op (~18 correct bits).
"""
import sys
sys.path.insert(0, "/opt/trn_rl_repo")

import numpy as np
import concourse.bass as bass
import concourse.tile as tile
from concourse import bacc, mybir
from concourse.bass_utils import run_bass_kernel_spmd

F32 = mybir.dt.float32
F32R = mybir.dt.float32r
AF = mybir.ActivationFunctionType
OP = mybir.AluOpType

B, C, H, W = 4, 128, 64, 64
N = H * W                      # 4096
NH, DH = 4, 32
EPS = 1e-6
NCH = 512                      # free-dim chunk (psum bank)
NJ = N // NCH                  # 8 n-chunks
MC = 128                       # m-chunk (partition tile)
NM = N // MC                   # 32 m-chunks
SCALE = DH ** -0.5


def build_nc(debug: bool = False, reps: int = 1):
    nc = bacc.Bacc("TRN2", target_bir_lowering=False)
    d_x = nc.dram_tensor("x", [C, N], F32, kind="ExternalInput")
    d_wqq = nc.dram_tensor("wqq", [C, 128], F32, kind="ExternalInput")
    d_wkk = nc.dram_tensor("wkk", [C, 128], F32, kind="ExternalInput")
    d_wv = nc.dram_tensor("wv", [C, 64], F32, kind="ExternalInput")
    d_bqq = nc.dram_tensor("bqq", [128, 1], F32, kind="ExternalInput")
    d_bkk = nc.dram_tensor("bkk", [128, 1], F32, kind="ExternalInput")
    d_bv = nc.dram_tensor("bv", [C, 64], F32, kind="ExternalInput")
    d_pw = nc.dram_tensor("pw", [65, C], F32, kind="ExternalInput")   # projT + bias row
    d_res = nc.dram_tensor("res", [C, 1], F32, kind="ExternalInput")  # residual scale col
    d_ind = nc.dram_tensor("ind", [2, 64], F32, kind="ExternalInput")  # head indicator
    d_out = nc.dram_tensor("out", [C, N], F32, kind="ExternalOutput")

    with tile.TileContext(nc) as tc:
        with tc.tile_pool(name="persist", bufs=1) as P:
            x_sb = P.tile([C, N], F32, tag="x_sb")
            xhat = P.tile([C, N], F32R, tag="xhat")
            qq2 = P.tile([C, N], F32R, tag="qq2")
            kk2 = P.tile([C, N], F32R, tag="kk2")
            vta = P.tile([C, NM, 66], F32R, tag="vta")   # per m-chunk: [v0|1|v1|1]
            w_qq = P.tile([C, 128], F32, tag="w_qq")
            w_kk = P.tile([C, 128], F32, tag="w_kk")
            w_v = P.tile([C, 64], F32, tag="w_v")
            wr_qq = P.tile([C, 128], F32R, tag="wr_qq")
            wr_kk = P.tile([C, 128], F32R, tag="wr_kk")
            wr_v = P.tile([C, 64], F32R, tag="wr_v")
            b_qq = P.tile([128, 1], F32, tag="b_qq")
            b_kk = P.tile([128, 1], F32, tag="b_kk")
            bv_b = P.tile([C, 64], F32, tag="bv_b")
            w_p = P.tile([65, C], F32, tag="w_p")
            wr_p = P.tile([65, C], F32R, tag="wr_p")
            res_c = P.tile([C, 1], F32, tag="res_c")
            ind2 = P.tile([2, 64], F32, tag="ind2")
            ones_m = P.tile([C, C], F32, tag="ones_m")
            ones_r = P.tile([C, C], F32R, tag="ones_r")
            eps_c = P.tile([C, 1], F32, tag="eps_c")
            ones_n = P.tile([1, NCH], F32, tag="ones_n")
            hn_a = P.tile([65, NCH], F32R, tag="hn_a")   # hn + ones row (proj bias)
            hn_b = P.tile([65, NCH], F32R, tag="hn_b")
            hbuf = [P.tile([97, NCH], F32, tag=f"hbuf{j}", name=f"hbuf{j}") for j in range(NJ)]
            rec01 = [P.tile([2, NCH], F32, tag=f"rec01_{j}", name=f"rec01_{j}") for j in range(NJ)]
            rec1t = [P.tile([1, NCH], F32, tag=f"rec1t_{j}", name=f"rec1t_{j}") for j in range(NJ)]

            nc.sync.dma_start(out=w_qq, in_=d_wqq.ap())
            nc.sync.dma_start(out=w_kk, in_=d_wkk.ap())
            nc.sync.dma_start(out=w_v, in_=d_wv.ap())
            nc.sync.dma_start(out=b_qq, in_=d_bqq.ap())
            nc.sync.dma_start(out=b_kk, in_=d_bkk.ap())
            nc.sync.dma_start(out=bv_b, in_=d_bv.ap())
            nc.sync.dma_start(out=w_p, in_=d_pw.ap())
            nc.sync.dma_start(out=res_c, in_=d_res.ap())
            nc.sync.dma_start(out=ind2, in_=d_ind.ap())
            nc.vector.memset(ones_m, 1.0)
            nc.vector.memset(eps_c, EPS)
            nc.vector.memset(ones_n, 1.0)
            nc.vector.tensor_copy(out=ones_r, in_=ones_m)
            nc.vector.tensor_copy(out=wr_qq, in_=w_qq)
            nc.vector.tensor_copy(out=wr_kk, in_=w_kk)
            nc.vector.tensor_copy(out=wr_v, in_=w_v)
            nc.vector.tensor_copy(out=wr_p, in_=w_p)
            nc.vector.tensor_copy(out=hn_a[64:65, :], in_=ones_n)
            nc.vector.tensor_copy(out=hn_b[64:65, :], in_=ones_n)
            nc.vector.tensor_copy(out=vta[:, :, 32:33], in_=ones_m[:, 0:NM])
            nc.vector.tensor_copy(out=vta[:, :, 65:66], in_=ones_m[:, 0:NM])
            for j in range(NJ):
                # rows 33:64 are never written but read by the batched recip
                nc.vector.memset(hbuf[j][32:64, :], 1.0)

            for rep in range(reps):
                # ---------- Phase 1+2: x load, LN stats, xhat, projections ----
                with tc.tile_pool(name="stats", bufs=3) as SP, \
                     tc.tile_pool(name="spool", bufs=2, space="PSUM") as SPOOL, \
                     tc.tile_pool(name="pvpool", bufs=1, space="PSUM") as PVP, \
                     tc.tile_pool(name="dscr", bufs=1, space="DRAM") as DSC, \
                     tc.tile_pool(name="ptpool", bufs=3) as PTP, \
                     tc.tile_pool(name="opool", bufs=2) as OPO, \
                     tc.tile_pool(name="npool", bufs=2) as NPO:
                    scr = [DSC.tile([2, NCH], F32, tag=f"scr{j}", name=f"scr{j}")
                           for j in range(NJ)]
                    def tail_a(j):
                        rb = NPO.tile([97, NCH], F32, tag="rb", name=f"rb{j}")
                        s0 = bass.AP(tensor=scr[j].tensor, offset=scr[j].offset,
                                     ap=[[0, 32], [1, NCH]])
                        s1 = bass.AP(tensor=scr[j].tensor,
                                     offset=scr[j].offset + NCH,
                                     ap=[[0, 32], [1, NCH]])
                        nc.sync.dma_start(out=rb[0:32, :], in_=s0)
                        nc.sync.dma_start(out=rb[64:96, :], in_=s1)
                        hn = hn_a if j % 2 == 0 else hn_b
                        nc.vector.tensor_mul(hn[0:32, :], hbuf[j][0:32, :],
                                             rb[0:32, :])
                        nc.vector.tensor_mul(hn[32:64, :], hbuf[j][64:96, :],
                                             rb[64:96, :])

                    def tail_b(j):
                        js = slice(j * NCH, (j + 1) * NCH)
                        hn = hn_a if j % 2 == 0 else hn_b
                        pj = SPOOL.tile([C, NCH], F32, tag="sg", name=f"pj{j}")
                        nc.tensor.matmul(pj, wr_p, hn, start=True, stop=True)
                        ot = OPO.tile([C, NCH], F32, tag="ot", name=f"ot{j}")
                        nc.vector.scalar_tensor_tensor(out=ot, in0=x_sb[:, js],
                                                       scalar=res_c, in1=pj,
                                                       op0=OP.mult, op1=OP.add)
                        nc.sync.dma_start(out=d_out.ap()[:, js], in_=ot)

                    pairs = [(i % 2, i // 2) for i in range(2 * NM)]
                    NG = (2 * NM + 2) // 3          # 22 groups per n-chunk

                    def attn_group(j, gi, pvs):
                        js = slice(j * NCH, (j + 1) * NCH)
                        g0 = 3 * gi
                        grp = pairs[g0:g0 + 3]
                        sg = SPOOL.tile([C, 3 * NCH], F32, tag="sg",
                                        name=f"sg{j}_{gi}")
                        seen = {0: 0, 1: 0}
                        for i, (h, mc) in enumerate(grp):
                            rg = h + 2 * seen[h]     # row-group 0..3
                            seen[h] += 1
                            ms = slice(mc * MC, (mc + 1) * MC)
                            rs = slice(rg * 32, (rg + 1) * 32)
                            nc.tensor.matmul(sg[:, i * NCH:(i + 1) * NCH],
                                             kk2[rs, ms], qq2[rs, js],
                                             start=True, stop=True,
                                             tile_position=(rg * 32, 0))
                        pt = PTP.tile([C, 3 * NCH], F32R, tag="pt",
                                      name=f"pt{j}_{gi}")
                        nw = len(grp) * NCH
                        nc.scalar.activation(out=pt[:, 0:nw], in_=sg[:, 0:nw],
                                             func=AF.Exp, scale=SCALE)
                        for i, (h, mc) in enumerate(grp):
                            pi = g0 + i
                            vcols = slice(33 * h, 33 * h + 33)
                            nc.tensor.matmul(pvs[h], vta[:, mc, vcols],
                                             pt[:, i * NCH:(i + 1) * NCH],
                                             start=(pi == h),
                                             stop=(pi == 2 * NM - 2 + h))

                    def attn_end(j, pvs):
                        pv0, pv1 = pvs
                        nc.vector.tensor_copy(out=hbuf[j][0:33, :],
                                              in_=pv0[0:33, :])
                        nc.vector.tensor_copy(out=hbuf[j][64:97, :],
                                              in_=pv1[0:33, :])
                        rcf = NPO.tile([97, NCH], F32, tag="rcf", name=f"rcf{j}")
                        nc.vector.reciprocal_approx_fast(out=rcf, in_=hbuf[j])
                        nc.sync.dma_start(out=scr[j][0:1, :], in_=rcf[32:33, :])
                        nc.sync.dma_start(out=scr[j][1:2, :], in_=rcf[96:97, :])


                    pv0_0 = PVP.tile([33, NCH], F32, tag="pv0", name="pv0_0")
                    pv1_0 = PVP.tile([33, NCH], F32, tag="pv1", name="pv1_0")
                    pvs0 = (pv0_0, pv1_0)
                    next_g = [0]

                    def stream_j0(c):
                        while next_g[0] < NG and \
                                min(3 * next_g[0] + 2, 2 * NM - 1) // 2 <= 4 * c + 3:
                            attn_group(0, next_g[0], pvs0)
                            next_g[0] += 1

                    mBs, invs, s12s = {}, {}, {}
                    for j in range(NJ):
                        js = slice(j * NCH, (j + 1) * NCH)
                        nc.sync.dma_start(out=x_sb[:, js], in_=d_x.ap()[:, js])
                        x2 = SP.tile([C, NCH], F32R, tag="x2", name=f"x2_{j}")
                        nc.vector.tensor_mul(x2, x_sb[:, js], x_sb[:, js])
                        s12 = SPOOL.tile([C, 2, NCH], F32, tag="sg", name=f"s12_{j}")
                        # all-ones lhsT => every out partition = sum over channels
                        nc.tensor.matmul(s12[:, 0, :], ones_m, x_sb[:, js],
                                         start=True, stop=True)
                        nc.tensor.matmul(s12[:, 1, :], ones_r, x2,
                                         start=True, stop=True)
                        mB = SP.tile([C, NCH], F32, tag="mB", name=f"mB_{j}")
                        nc.vector.tensor_scalar(out=mB, in0=s12[:, 0, :],
                                                scalar1=1.0 / C,
                                                scalar2=None, op0=OP.mult)
                        mBs[j] = mB
                        s12s[j] = s12[:, 1, :]
                        msq = SP.tile([C, NCH], F32, tag="msq", name=f"msq_{j}")
                        nc.vector.tensor_mul(msq, mB, mB)
                        var = SP.tile([C, NCH], F32, tag="var", name=f"var_{j}")
                        nc.vector.scalar_tensor_tensor(out=var, in0=s12s[j], scalar=1.0 / C,
                                                       in1=msq, op0=OP.mult,
                                                       op1=OP.subtract)
                        sd = SP.tile([C, NCH], F32, tag="sd", name=f"sd_{j}")
                        nc.scalar.activation(out=sd, in_=var, func=AF.Sqrt,
                                             bias=eps_c, scale=1.0)
                        inv = SP.tile([C, NCH], F32, tag="inv", name=f"inv_{j}")
                        nc.vector.reciprocal_approx_fast(out=inv, in_=sd)
                        invs[j] = inv
                    for j in range(NJ):
                        js = slice(j * NCH, (j + 1) * NCH)
                        cen = SP.tile([C, NCH], F32, tag="cen", name=f"cen_{j}")
                        nc.vector.tensor_sub(cen, x_sb[:, js], mBs[j])
                        nc.vector.tensor_mul(xhat[:, js], cen, invs[j])
                        # projections for this chunk
                        qkp = SPOOL.tile([C, 2, NCH], F32, tag="sg", name=f"qkp{j}")
                        nc.tensor.matmul(qkp[:, 0, :], wr_qq, xhat[:, js],
                                         start=True, stop=True)
                        nc.vector.tensor_scalar(out=qq2[:, js], in0=qkp[:, 0, :],
                                                scalar1=b_qq,
                                                scalar2=None, op0=OP.add)
                        nc.tensor.matmul(qkp[:, 1, :], wr_kk, xhat[:, js],
                                         start=True, stop=True)
                        nc.vector.tensor_scalar(out=kk2[:, js], in0=qkp[:, 1, :],
                                                scalar1=b_kk,
                                                scalar2=None, op0=OP.add)
                        vpq = SPOOL.tile([C, 4, 64], F32, tag="sg", name=f"vpq{j}")
                        for mq in range(4):
                            mc = 4 * j + mq
                            ms = slice(mc * MC, (mc + 1) * MC)
                            nc.tensor.matmul(vpq[:, mq, :], xhat[:, ms], wr_v,
                                             start=True, stop=True)
                            vdst = vta[:, mc, 0:66].rearrange(
                                "p (a b) -> p a b", a=2)[:, :, 0:32]
                            vsrc = vpq[:, mq, :].rearrange("p (a b) -> p a b", a=2)
                            bsrc = bv_b.rearrange("p (a b) -> p a b", a=2)
                            nc.vector.tensor_add(vdst, vsrc, bsrc)

                    stream_j0(NJ - 1)
                    attn_end(0, pvs0)
                    for j in range(1, NJ):
                        pv0 = PVP.tile([33, NCH], F32, tag="pv0", name=f"pv0_{j}")
                        pv1 = PVP.tile([33, NCH], F32, tag="pv1", name=f"pv1_{j}")
                        for gi in range(NG):
                            if gi == 3:
                                tail_a(j - 1)
                            if gi == 9:
                                tail_b(j - 1)
                            attn_group(j, gi, (pv0, pv1))
                        attn_end(j, (pv0, pv1))
                    tail_a(NJ - 1)
                    tail_b(NJ - 1)
    nc.compile()
    return nc


def _prep_inputs(x, norm_w, norm_b, qkv_w, qkv_b, proj_w, proj_b):
    """Host-side fold + per-core slicing. Returns list of 8 in_maps."""
    xf = np.ascontiguousarray(x.reshape(B, C, N), dtype=np.float32)
    qkv_wf = (qkv_w * norm_w[None, :]).astype(np.float32)
    qkv_bf = (qkv_b + qkv_w @ norm_b).astype(np.float32)
    in_maps = []
    for core in range(8):
        b, hp = core // 2, core % 2
        h0, h1 = 2 * hp, 2 * hp + 1
        qrows = list(range(h0 * DH, h0 * DH + DH)) + list(range(h1 * DH, h1 * DH + DH))
        krows = [C + r for r in qrows]
        vrows = [2 * C + r for r in qrows]
        qrows2 = qrows + qrows                           # duplicated for row-packing
        krows2 = krows + krows
        wqq = qkv_wf[qrows2, :].T.copy()                 # [C, 128]
        wkk = qkv_wf[krows2, :].T.copy()
        wv = qkv_wf[vrows, :].T.copy()                   # [C, 64]
        bqq = qkv_bf[qrows2].reshape(128, 1).copy()
        bkk = qkv_bf[krows2].reshape(128, 1).copy()
        bv = np.broadcast_to(qkv_bf[vrows].reshape(1, 64), (C, 64)).copy()
        cols = qrows
        pw = np.zeros((65, C), np.float32)
        pw[0:64, :] = proj_w[:, cols].T
        if hp == 0:
            pw[64, :] = proj_b
        res = np.full((C, 1), 1.0 if hp == 0 else 0.0, np.float32)
        ind = np.zeros((2, 64), np.float32)
        ind[0, 0:32] = 1.0
        ind[1, 32:64] = 1.0
        in_maps.append({
            "x": np.ascontiguousarray(xf[b]), "wqq": wqq, "wkk": wkk, "wv": wv,
            "bqq": bqq, "bkk": bkk, "bv": bv, "pw": pw, "res": res, "ind": ind,
        })
    return in_maps


_NC_CACHE = None


def kernel(x, norm_w, norm_b, qkv_w, qkv_b, proj_w, proj_b, **extra):
    global _NC_CACHE
    x = np.asarray(x, dtype=np.float32)
    in_maps = _prep_inputs(x, np.asarray(norm_w), np.asarray(norm_b),
                           np.asarray(qkv_w), np.asarray(qkv_b),
                           np.asarray(proj_w), np.asarray(proj_b))
    if _NC_CACHE is None:
        _NC_CACHE = build_nc()
    res = run_bass_kernel_spmd(_NC_CACHE, in_maps, core_ids=list(range(8)))
    parts = [res.results[i]["out"] for i in range(8)]
    out = np.empty((B, C, N), np.float32)
    for b in range(B):
        out[b] = parts[2 * b] + parts[2 * b + 1]
    return out.reshape(B, C, H, W)


if __name__ == "__main__":
    rng = np.random.default_rng(0)
    x = rng.standard_normal((B, C, H, W)).astype(np.float32)
    nw = np.ones(C, np.float32)
    nb = np.zeros(C, np.float32)
    qw = (rng.standard_normal((3 * C, C)) / np.sqrt(C)).astype(np.float32)
    qb = np.zeros(3 * C, np.float32)
    pw = (rng.standard_normal((C, C)) / np.sqrt(C)).astype(np.float32)
    pb = np.zeros(C, np.float32)
    got = kernel(x, nw, nb, qw, qb, pw, pb)
    print("kernel ran, shape", got.shape)


# revision 21
# speedup vs baseline: 1.4379x; 1.0125x over previous
"""Trainium2 Bass kernel for nn_Attention_39651138076722.

ChannelLayerNorm -> qkv 1x1 conv -> 4-head spatial attention (N=4096, dh=32)
-> proj 1x1 conv -> residual.   B=4, C=128, H=W=64.

Sharding: 8 cores = 4 batches x 2 head-pairs. Each core computes the partial
proj output of its 2 heads for its batch; the host sums the two partials.
LayerNorm affine (norm_w/norm_b) is folded into the qkv weights on the host.
Big matmuls run as float32r (~1.6e-4 max rel err, 4x faster than fp32).

Attention works on S^T = k.T q tiles [m=128, n=512]: four K=32 matmuls are
row-packed into the PE array per group (2 heads x 2 m-chunks, via
row-duplicated qq2/kk2 layouts), one big exp on ACT per 4-bank PSUM group,
and PV accumulates h rows + a ones-row (softmax denominator) per head.
Normalization + proj run as a deferred tail pass over saved h/denom tiles.
Reciprocals use the single-pass approx-fast DVE op (~18 correct bits).
"""
import sys
sys.path.insert(0, "/opt/trn_rl_repo")

import numpy as np
import concourse.bass as bass
import concourse.tile as tile
from concourse import bacc, mybir
from concourse.bass_utils import run_bass_kernel_spmd

F32 = mybir.dt.float32
F32R = mybir.dt.float32r
BF16 = mybir.dt.bfloat16
AF = mybir.ActivationFunctionType
OP = mybir.AluOpType

B, C, H, W = 4, 128, 64, 64
N = H * W                      # 4096
NH, DH = 4, 32
EPS = 1e-6
NCH = 512                      # free-dim chunk (psum bank)
NJ = N // NCH                  # 8 n-chunks
MC = 128                       # m-chunk (partition tile)
NM = N // MC                   # 32 m-chunks
SCALE = DH ** -0.5
import math
# Schraudolph exp -> bf16 bits: low 2 bytes of fp32(A16*S + MAGIC16) hold the
# bf16 bit pattern of exp(S*SCALE)  (sigma=9.6 calibrated for softmax use)
A16 = 128.0 * math.log2(math.e) * SCALE
MAGIC16 = 12582912.0 + (16256.0 - 9.6)


def build_nc(debug: bool = False, reps: int = 1):
    nc = bacc.Bacc("TRN2", target_bir_lowering=False)
    d_x = nc.dram_tensor("x", [C, N], F32, kind="ExternalInput")
    d_wqq = nc.dram_tensor("wqq", [C, 128], F32, kind="ExternalInput")
    d_wkk = nc.dram_tensor("wkk", [C, 128], F32, kind="ExternalInput")
    d_wv = nc.dram_tensor("wv", [C, 64], F32, kind="ExternalInput")
    d_bqq = nc.dram_tensor("bqq", [128, 1], F32, kind="ExternalInput")
    d_bkk = nc.dram_tensor("bkk", [128, 1], F32, kind="ExternalInput")
    d_bv = nc.dram_tensor("bv", [C, 64], F32, kind="ExternalInput")
    d_pw = nc.dram_tensor("pw", [65, C], F32, kind="ExternalInput")   # projT + bias row
    d_res = nc.dram_tensor("res", [C, 1], F32, kind="ExternalInput")  # residual scale col
    d_ind = nc.dram_tensor("ind", [2, 64], F32, kind="ExternalInput")  # head indicator
    d_out = nc.dram_tensor("out", [C, N], F32, kind="ExternalOutput")

    with tile.TileContext(nc) as tc:
        with tc.tile_pool(name="persist", bufs=1) as P:
            x_sb = P.tile([C, N], F32, tag="x_sb")
            xhat = P.tile([C, N], F32R, tag="xhat")
            qq2 = P.tile([C, N], F32R, tag="qq2")
            kk2 = P.tile([C, N], F32R, tag="kk2")
            vta = P.tile([C, NM, 66], BF16, tag="vta")   # per m-chunk: [v0|1|v1|1]
            w_qq = P.tile([C, 128], F32, tag="w_qq")
            w_kk = P.tile([C, 128], F32, tag="w_kk")
            w_v = P.tile([C, 64], F32, tag="w_v")
            wr_qq = P.tile([C, 128], F32R, tag="wr_qq")
            wr_kk = P.tile([C, 128], F32R, tag="wr_kk")
            wr_v = P.tile([C, 64], F32R, tag="wr_v")
            b_qq = P.tile([128, 1], F32, tag="b_qq")
            b_kk = P.tile([128, 1], F32, tag="b_kk")
            bv_b = P.tile([C, 64], F32, tag="bv_b")
            w_p = P.tile([65, C], F32, tag="w_p")
            wr_p = P.tile([65, C], F32R, tag="wr_p")
            res_c = P.tile([C, 1], F32, tag="res_c")
            ind2 = P.tile([2, 64], F32, tag="ind2")
            ones_m = P.tile([C, C], F32, tag="ones_m")
            ones_r = P.tile([C, C], F32R, tag="ones_r")
            eps_c = P.tile([C, 1], F32, tag="eps_c")
            ones_n = P.tile([1, NCH], F32, tag="ones_n")
            hn_a = P.tile([65, NCH], F32R, tag="hn_a")   # hn + ones row (proj bias)
            hn_b = P.tile([65, NCH], F32R, tag="hn_b")
            hbuf = [P.tile([97, NCH], F32, tag=f"hbuf{j}", name=f"hbuf{j}") for j in range(NJ)]
            rec01 = [P.tile([2, NCH], F32, tag=f"rec01_{j}", name=f"rec01_{j}") for j in range(NJ)]
            rec1t = [P.tile([1, NCH], F32, tag=f"rec1t_{j}", name=f"rec1t_{j}") for j in range(NJ)]

            nc.sync.dma_start(out=w_qq, in_=d_wqq.ap())
            nc.sync.dma_start(out=w_kk, in_=d_wkk.ap())
            nc.sync.dma_start(out=w_v, in_=d_wv.ap())
            nc.sync.dma_start(out=b_qq, in_=d_bqq.ap())
            nc.sync.dma_start(out=b_kk, in_=d_bkk.ap())
            nc.sync.dma_start(out=bv_b, in_=d_bv.ap())
            nc.sync.dma_start(out=w_p, in_=d_pw.ap())
            nc.sync.dma_start(out=res_c, in_=d_res.ap())
            nc.sync.dma_start(out=ind2, in_=d_ind.ap())
            nc.vector.memset(ones_m, 1.0)
            nc.vector.memset(eps_c, EPS)
            nc.vector.memset(ones_n, 1.0)
            nc.vector.tensor_copy(out=ones_r, in_=ones_m)
            nc.vector.tensor_copy(out=wr_qq, in_=w_qq)
            nc.vector.tensor_copy(out=wr_kk, in_=w_kk)
            nc.vector.tensor_copy(out=wr_v, in_=w_v)
            nc.vector.tensor_copy(out=wr_p, in_=w_p)
            nc.vector.tensor_copy(out=hn_a[64:65, :], in_=ones_n)
            nc.vector.tensor_copy(out=hn_b[64:65, :], in_=ones_n)
            nc.vector.tensor_copy(out=vta[:, :, 32:33], in_=ones_m[:, 0:NM])
            nc.vector.tensor_copy(out=vta[:, :, 65:66], in_=ones_m[:, 0:NM])
            for j in range(NJ):
                # rows 33:64 are never written but read by the batched recip
                nc.vector.memset(hbuf[j][32:64, :], 1.0)

            for rep in range(reps):
                # ---------- Phase 1+2: x load, LN stats, xhat, projections ----
                with tc.tile_pool(name="stats", bufs=3) as SP, \
                     tc.tile_pool(name="spool", bufs=2, space="PSUM") as SPOOL, \
                     tc.tile_pool(name="pvpool", bufs=1, space="PSUM") as PVP, \
                     tc.tile_pool(name="dscr", bufs=1, space="DRAM") as DSC, \
                     tc.tile_pool(name="ptpool", bufs=3) as PTP, \
                     tc.tile_pool(name="ptdpool", bufs=2) as PTD, \
                     tc.tile_pool(name="opool", bufs=2) as OPO, \
                     tc.tile_pool(name="npool", bufs=2) as NPO:
                    scr = [DSC.tile([2, NCH], F32, tag=f"scr{j}", name=f"scr{j}")
                           for j in range(NJ)]
                    def tail_a(j):
                        rb = NPO.tile([97, NCH], F32, tag="rb", name=f"rb{j}")
                        s0 = bass.AP(tensor=scr[j].tensor, offset=scr[j].offset,
                                     ap=[[0, 32], [1, NCH]])
                        s1 = bass.AP(tensor=scr[j].tensor,
                                     offset=scr[j].offset + NCH,
                                     ap=[[0, 32], [1, NCH]])
                        nc.sync.dma_start(out=rb[0:32, :], in_=s0)
                        nc.sync.dma_start(out=rb[64:96, :], in_=s1)
                        hn = hn_a if j % 2 == 0 else hn_b
                        nc.vector.tensor_mul(hn[0:32, :], hbuf[j][0:32, :],
                                             rb[0:32, :])
                        nc.vector.tensor_mul(hn[32:64, :], hbuf[j][64:96, :],
                                             rb[64:96, :])

                    def tail_b(j):
                        js = slice(j * NCH, (j + 1) * NCH)
                        hn = hn_a if j % 2 == 0 else hn_b
                        pj = SPOOL.tile([C, NCH], F32, tag="sg", name=f"pj{j}")
                        nc.tensor.matmul(pj, wr_p, hn, start=True, stop=True)
                        ot = OPO.tile([C, NCH], F32, tag="ot", name=f"ot{j}")
                        nc.vector.scalar_tensor_tensor(out=ot, in0=x_sb[:, js],
                                                       scalar=res_c, in1=pj,
                                                       op0=OP.mult, op1=OP.add)
                        nc.sync.dma_start(out=d_out.ap()[:, js], in_=ot)

                    pairs = [(i % 2, i // 2) for i in range(2 * NM)]
                    NG = (2 * NM + 2) // 3          # 22 groups per n-chunk

                    def attn_group(j, gi, pvs):
                        js = slice(j * NCH, (j + 1) * NCH)
                        g0 = 3 * gi
                        grp = pairs[g0:g0 + 3]
                        sg = SPOOL.tile([C, 3 * NCH], F32, tag="sg",
                                        name=f"sg{j}_{gi}")
                        seen = {0: 0, 1: 0}
                        for i, (h, mc) in enumerate(grp):
                            rg = h + 2 * seen[h]     # row-group 0..3
                            seen[h] += 1
                            ms = slice(mc * MC, (mc + 1) * MC)
                            rs = slice(rg * 32, (rg + 1) * 32)
                            nc.tensor.matmul(sg[:, i * NCH:(i + 1) * NCH],
                                             kk2[rs, ms], qq2[rs, js],
                                             start=True, stop=True,
                                             tile_position=(rg * 32, 0))
                        nw = len(grp) * NCH
                        if gi % 3 == 2:
                            ptd = PTD.tile([C, 3 * NCH], F32, tag="ptd",
                                           name=f"ptd{j}_{gi}")
                            nc.vector.tensor_scalar(out=ptd[:, 0:nw],
                                                    in0=sg[:, 0:nw],
                                                    scalar1=A16, scalar2=MAGIC16,
                                                    op0=OP.mult, op1=OP.add)
                            ptv = ptd[:, :].bitcast(BF16).rearrange(
                                "p (n f) -> p n f", f=2)
                            rhss = [ptv[:, i * NCH:(i + 1) * NCH, 0]
                                    for i in range(len(grp))]
                        else:
                            pt = PTP.tile([C, 3 * NCH], BF16, tag="pt",
                                          name=f"pt{j}_{gi}")
                            nc.scalar.activation(out=pt[:, 0:nw], in_=sg[:, 0:nw],
                                                 func=AF.Exp, scale=SCALE)
                            rhss = [pt[:, i * NCH:(i + 1) * NCH]
                                    for i in range(len(grp))]
                        for i, (h, mc) in enumerate(grp):
                            pi = g0 + i
                            vcols = slice(33 * h, 33 * h + 33)
                            nc.tensor.matmul(pvs[h], vta[:, mc, vcols],
                                             rhss[i],
                                             start=(pi == h),
                                             stop=(pi == 2 * NM - 2 + h))

                    def attn_end(j, pvs):
                        pv0, pv1 = pvs
                        nc.vector.tensor_copy(out=hbuf[j][0:33, :],
                                              in_=pv0[0:33, :])
                        nc.vector.tensor_copy(out=hbuf[j][64:97, :],
                                              in_=pv1[0:33, :])
                        rcf = NPO.tile([97, NCH], F32, tag="rcf", name=f"rcf{j}")
                        nc.vector.reciprocal_approx_fast(out=rcf, in_=hbuf[j])
                        nc.sync.dma_start(out=scr[j][0:1, :], in_=rcf[32:33, :])
                        nc.sync.dma_start(out=scr[j][1:2, :], in_=rcf[96:97, :])


                    pv0_0 = PVP.tile([33, NCH], F32, tag="pv0", name="pv0_0")
                    pv1_0 = PVP.tile([33, NCH], F32, tag="pv1", name="pv1_0")
                    pvs0 = (pv0_0, pv1_0)
                    next_g = [0]

                    def stream_j0(c):
                        while next_g[0] < NG and \
                                min(3 * next_g[0] + 2, 2 * NM - 1) // 2 <= 4 * c + 3:
                            attn_group(0, next_g[0], pvs0)
                            next_g[0] += 1

                    mBs, invs, s12s = {}, {}, {}
                    for j in range(NJ):
                        js = slice(j * NCH, (j + 1) * NCH)
                        nc.sync.dma_start(out=x_sb[:, js], in_=d_x.ap()[:, js])
                        x2 = SP.tile([C, NCH], F32R, tag="x2", name=f"x2_{j}")
                        nc.vector.tensor_mul(x2, x_sb[:, js], x_sb[:, js])
                        s12 = SPOOL.tile([C, 2, NCH], F32, tag="sg", name=f"s12_{j}")
                        # all-ones lhsT => every out partition = sum over channels
                        nc.tensor.matmul(s12[:, 0, :], ones_m, x_sb[:, js],
                                         start=True, stop=True)
                        nc.tensor.matmul(s12[:, 1, :], ones_r, x2,
                                         start=True, stop=True)
                        mB = SP.tile([C, NCH], F32, tag="mB", name=f"mB_{j}")
                        nc.vector.tensor_scalar(out=mB, in0=s12[:, 0, :],
                                                scalar1=1.0 / C,
                                                scalar2=None, op0=OP.mult)
                        mBs[j] = mB
                        s12s[j] = s12[:, 1, :]
                        msq = SP.tile([C, NCH], F32, tag="msq", name=f"msq_{j}")
                        nc.vector.tensor_mul(msq, mB, mB)
                        var = SP.tile([C, NCH], F32, tag="var", name=f"var_{j}")
                        nc.vector.scalar_tensor_tensor(out=var, in0=s12s[j], scalar=1.0 / C,
                                                       in1=msq, op0=OP.mult,
                                                       op1=OP.subtract)
                        sd = SP.tile([C, NCH], F32, tag="sd", name=f"sd_{j}")
                        nc.scalar.activation(out=sd, in_=var, func=AF.Sqrt,
                                             bias=eps_c, scale=1.0)
                        inv = SP.tile([C, NCH], F32, tag="inv", name=f"inv_{j}")
                        nc.vector.reciprocal_approx_fast(out=inv, in_=sd)
                        invs[j] = inv
                    for j in range(NJ):
                        js = slice(j * NCH, (j + 1) * NCH)
                        cen = SP.tile([C, NCH], F32, tag="cen", name=f"cen_{j}")
                        nc.vector.tensor_sub(cen, x_sb[:, js], mBs[j])
                        nc.vector.tensor_mul(xhat[:, js], cen, invs[j])
                        # projections for this chunk
                        qkp = SPOOL.tile([C, 2, NCH], F32, tag="sg", name=f"qkp{j}")
                        nc.tensor.matmul(qkp[:, 0, :], wr_qq, xhat[:, js],
                                         start=True, stop=True)
                        nc.vector.tensor_scalar(out=qq2[:, js], in0=qkp[:, 0, :],
                                                scalar1=b_qq,
                                                scalar2=None, op0=OP.add)
                        nc.tensor.matmul(qkp[:, 1, :], wr_kk, xhat[:, js],
                                         start=True, stop=True)
                        nc.vector.tensor_scalar(out=kk2[:, js], in0=qkp[:, 1, :],
                                                scalar1=b_kk,
                                                scalar2=None, op0=OP.add)
                        vpq = SPOOL.tile([C, 4, 64], F32, tag="sg", name=f"vpq{j}")
                        for mq in range(4):
                            mc = 4 * j + mq
                            ms = slice(mc * MC, (mc + 1) * MC)
                            nc.tensor.matmul(vpq[:, mq, :], xhat[:, ms], wr_v,
                                             start=True, stop=True)
                            vdst = vta[:, mc, 0:66].rearrange(
                                "p (a b) -> p a b", a=2)[:, :, 0:32]
                            vsrc = vpq[:, mq, :].rearrange("p (a b) -> p a b", a=2)
                            bsrc = bv_b.rearrange("p (a b) -> p a b", a=2)
                            nc.vector.tensor_add(vdst, vsrc, bsrc)

                    stream_j0(NJ - 1)
                    attn_end(0, pvs0)
                    for j in range(1, NJ):
                        pv0 = PVP.tile([33, NCH], F32, tag="pv0", name=f"pv0_{j}")
                        pv1 = PVP.tile([33, NCH], F32, tag="pv1", name=f"pv1_{j}")
                        for gi in range(NG):
                            if gi == 3:
                                tail_a(j - 1)
                            if gi == 9:
                                tail_b(j - 1)
                            attn_group(j, gi, (pv0, pv1))
                        attn_end(j, (pv0, pv1))
                    tail_a(NJ - 1)
                    tail_b(NJ - 1)
    nc.compile()
    return nc


def _prep_inputs(x, norm_w, norm_b, qkv_w, qkv_b, proj_w, proj_b):
    """Host-side fold + per-core slicing. Returns list of 8 in_maps."""
    xf = np.ascontiguousarray(x.reshape(B, C, N), dtype=np.float32)
    qkv_wf = (qkv_w * norm_w[None, :]).astype(np.float32)
    qkv_bf = (qkv_b + qkv_w @ norm_b).astype(np.float32)
    in_maps = []
    for core in range(8):
        b, hp = core // 2, core % 2
        h0, h1 = 2 * hp, 2 * hp + 1
        qrows = list(range(h0 * DH, h0 * DH + DH)) + list(range(h1 * DH, h1 * DH + DH))
        krows = [C + r for r in qrows]
        vrows = [2 * C + r for r in qrows]
        qrows2 = qrows + qrows                           # duplicated for row-packing
        krows2 = krows + krows
        wqq = qkv_wf[qrows2, :].T.copy()                 # [C, 128]
        wkk = qkv_wf[krows2, :].T.copy()
        wv = qkv_wf[vrows, :].T.copy()                   # [C, 64]
        bqq = qkv_bf[qrows2].reshape(128, 1).copy()
        bkk = qkv_bf[krows2].reshape(128, 1).copy()
        bv = np.broadcast_to(qkv_bf[vrows].reshape(1, 64), (C, 64)).copy()
        cols = qrows
        pw = np.zeros((65, C), np.float32)
        pw[0:64, :] = proj_w[:, cols].T
        if hp == 0:
            pw[64, :] = proj_b
        res = np.full((C, 1), 1.0 if hp == 0 else 0.0, np.float32)
        ind = np.zeros((2, 64), np.float32)
        ind[0, 0:32] = 1.0
        ind[1, 32:64] = 1.0
        in_maps.append({
            "x": np.ascontiguousarray(xf[b]), "wqq": wqq, "wkk": wkk, "wv": wv,
            "bqq": bqq, "bkk": bkk, "bv": bv, "pw": pw, "res": res, "ind": ind,
        })
    return in_maps


_NC_CACHE = None


def kernel(x, norm_w, norm_b, qkv_w, qkv_b, proj_w, proj_b, **extra):
    global _NC_CACHE
    x = np.asarray(x, dtype=np.float32)
    in_maps = _prep_inputs(x, np.asarray(norm_w), np.asarray(norm_b),
                           np.asarray(qkv_w), np.asarray(qkv_b),
                           np.asarray(proj_w), np.asarray(proj_b))
    if _NC_CACHE is None:
        _NC_CACHE = build_nc()
    res = run_bass_kernel_spmd(_NC_CACHE, in_maps, core_ids=list(range(8)))
    parts = [res.results[i]["out"] for i in range(8)]
    out = np.empty((B, C, N), np.float32)
    for b in range(B):
        out[b] = parts[2 * b] + parts[2 * b + 1]
    return out.reshape(B, C, H, W)


if __name__ == "__main__":
    rng = np.random.default_rng(0)
    x = rng.standard_normal((B, C, H, W)).astype(np.float32)
    nw = np.ones(C, np.float32)
    nb = np.zeros(C, np.float32)
    qw = (rng.standard_normal((3 * C, C)) / np.sqrt(C)).astype(np.float32)
    qb = np.zeros(3 * C, np.float32)
    pw = (rng.standard_normal((C, C)) / np.sqrt(C)).astype(np.float32)
    pb = np.zeros(C, np.float32)
    got = kernel(x, nw, nb, qw, qb, pw, pb)
    print("kernel ran, shape", got.shape)
